# revision 5
# baseline (speedup 1.0000x reference)
"""Bass/Trainium2 kernel for nn_MultiHeadAttention (T5-style rel-bias causal MHA).

Sharding: 8 cores = 2 batches x 4 head-groups (4 heads of 64 dims each).
Each core: projects q/k/v for its 256 proj rows (bf16 operands, fp32 PSUM),
runs causal attention with the T5 relative bias folded in as either a PSUM
band preload (near-diagonal blocks) or a constant exp-bias
(bucket-31-saturated blocks), and computes a partial out-projection.
Host sums the 4 bf16 partials per batch.

v2 layout: bf16 operands everywhere (halves HBM traffic vs f32r), m-tile
ordered q/k projection so attention sections for heads 0/1 start early
(their AV deferred until v-proj lands), qs-outer section order with the
qs0 out-projection overlapped under qs1 attention, and the scalar engine
kept free of DMA issue so it does nothing but exp.
"""
import math
import sys

sys.path.insert(0, "/opt/trn_rl_repo")

import numpy as np
import ml_dtypes

from concourse import bacc
import concourse.mybir as mybir
import concourse.tile as tile
from concourse.bass_utils import run_bass_kernel_spmd

F32 = mybir.dt.float32
BF = mybir.dt.bfloat16
Exp = mybir.ActivationFunctionType.Exp
Copy = mybir.ActivationFunctionType.Copy
MUL = mybir.AluOpType.mult

B, L, D = 2, 2048, 1024
H, HD = 16, 64
NUM_BUCKETS, MAX_DISTANCE = 32, 128
HPC = 4  # heads per core
MPC = HPC * HD  # 256 proj rows per core
N_CORES = 8
NEG = -60.0  # additive mask value (exp(-60+s) == 0 in practice)
BF_NP = ml_dtypes.bfloat16

last_results = None  # BassKernelResults of the most recent run (for profiling)
_cached = None


def _bucket(rp: np.ndarray) -> np.ndarray:
    """T5 relative position bucket, mirrors the reference exactly."""
    sign = (rp > 0).astype(np.int32)
    n = np.abs(rp)
    max_exact = NUM_BUCKETS // 2
    n_safe = np.maximum(n, 1).astype(np.float32)
    vil = max_exact + (
        np.log(n_safe / max_exact)
        / math.log(MAX_DISTANCE / max_exact)
        * (NUM_BUCKETS - max_exact)
    ).astype(np.int32)
    vil = np.minimum(vil, NUM_BUCKETS - 1)
    buckets = np.where(n < max_exact, n, vil) + sign * max_exact
    return np.clip(buckets, 0, NUM_BUCKETS - 1)


def _build():
    nc = bacc.Bacc(trn_type="TRN2")

    qT_in = nc.dram_tensor("qT_in", [D, L], BF, kind="ExternalInput")
    kT_in = nc.dram_tensor("kT_in", [D, L], BF, kind="ExternalInput")
    vT_in = nc.dram_tensor("vT_in", [D, L], BF, kind="ExternalInput")
    wq_in = nc.dram_tensor("wq_in", [128, 8, MPC], BF, kind="ExternalInput")
    wk_in = nc.dram_tensor("wk_in", [128, 8, MPC], BF, kind="ExternalInput")
    wv_in = nc.dram_tensor("wv_in", [128, 8, MPC], BF, kind="ExternalInput")
    wo_in = nc.dram_tensor("wo_in", [128, 2, D], BF, kind="ExternalInput")
    bq_in = nc.dram_tensor("bq_in", [128, 2], F32, kind="ExternalInput")
    bk_in = nc.dram_tensor("bk_in", [128, 2], F32, kind="ExternalInput")
    band_in = nc.dram_tensor("band_in", [HPC, 128, 4096], BF, kind="ExternalInput")
    c31_in = nc.dram_tensor("c31_in", [128, HPC], F32, kind="ExternalInput")
    id_in = nc.dram_tensor("id_in", [128, 128], BF, kind="ExternalInput")
    outT = nc.dram_tensor("outT", [D, L], BF, kind="ExternalOutput")

    with tile.TileContext(nc) as tc:
        with (
            tc.tile_pool(name="res", bufs=1) as pr,
            tc.tile_pool(name="qkv", bufs=1) as pqkv,
            tc.tile_pool(name="stg", bufs=16) as pstg,
            tc.tile_pool(name="es", bufs=16) as pes,
            tc.tile_pool(name="misc", bufs=2) as pmisc,
            tc.tile_pool(name="spsum", bufs=2, space="PSUM") as psc,
        ):
            # ---- input DMAs: qT chunks on sync, everything else gpsimd ----
            stq = []
            for kc in range(8):
                t = pstg.tile([128, L], BF, tag="stage", name=f"sq{kc}")
                nc.sync.dma_start(t[:], qT_in[128 * kc : 128 * kc + 128, :])
                stq.append(t)
            stk = []
            for kc in range(8):
                t = pstg.tile([128, L], BF, tag="stage", name=f"sk{kc}")
                nc.gpsimd.dma_start(t[:], kT_in[128 * kc : 128 * kc + 128, :])
                stk.append(t)
            wq = pr.tile([128, 8, MPC], BF)
            nc.gpsimd.dma_start(wq[:], wq_in[:])
            wk = pr.tile([128, 8, MPC], BF)
            nc.gpsimd.dma_start(wk[:], wk_in[:])
            bq = pr.tile([128, 2], F32)
            nc.gpsimd.dma_start(bq[:], bq_in[:])
            bk = pr.tile([128, 2], F32)
            nc.gpsimd.dma_start(bk[:], bk_in[:])
            c31 = pr.tile([128, HPC], F32)
            nc.gpsimd.dma_start(c31[:], c31_in[:])
            ident = pr.tile([128, 128], BF)
            nc.gpsimd.dma_start(ident[:], id_in[:])
            bands = []
            for h in range(HPC):
                t = pr.tile([128, 4096], BF, name=f"band{h}")
                nc.gpsimd.dma_start(t[:], band_in[h])
                bands.append(t)
            # v input staged behind q (reuses q's stage bufs), issued on sync
            stv = []
            for kc in range(8):
                t = pstg.tile([128, L], BF, tag="stage", name=f"sv{kc}")
                nc.sync.dma_start(t[:], vT_in[128 * kc : 128 * kc + 128, :])
                stv.append(t)
            wv = pr.tile([128, 8, MPC], BF)
            nc.gpsimd.dma_start(wv[:], wv_in[:])
            wo = pr.tile([128, 2, D], BF)
            nc.gpsimd.dma_start(wo[:], wo_in[:])

            # warm the ACT exp table early, off the critical path
            warm = pr.tile([1, 2], F32)
            nc.vector.memset(warm[:], 0.0)
            nc.scalar.activation(warm[:], warm[:], Exp)
            ones_v = pr.tile([1, HD], F32)
            nc.vector.memset(ones_v[:], 1.0)

            qTz = []
            for hh in range(HPC):
                t = pqkv.tile([128, L], BF, name=f"qtz{hh}")
                nc.vector.memset(t[:].bitcast(F32), 0)
                qTz.append(t)
            kTt = [pqkv.tile([128, L], BF, name=f"kt{mm}") for mm in range(2)]
            vxg = []
            for g in range(4):
                t = pqkv.tile([128, 4, HPC, HD + 1], BF, name=f"vx{g}")
                nc.vector.memset(t[:, :, :, HD], 1.0)
                vxg.append(t)
            y_norm_qs = [
                pqkv.tile([128, 2, 1024], BF, name=f"yn{qq}") for qq in range(2)
            ]

            # ---------------- attention section machinery ----------------
            pending_norm = [None]

            def _emit_norm(item):
                # PE-side replication of the reciprocal row + in-place
                # multiply; prep rides the score-psum ring.
                rrow, pb, mt, qsi = item
                prep = psc.tile([128, 1024], F32, tag="score", name="prep")
                nc.tensor.matmul(
                    prep[0:HD, :512], ones_v[:], rrow[:, :512],
                    start=True, stop=True,
                )
                nc.tensor.matmul(
                    prep[0:HD, 512:], ones_v[:], rrow[:, 512:],
                    start=True, stop=True,
                )
                prep_sb = pmisc.tile([128, 1024], BF, tag="prep")
                nc.vector.tensor_copy(prep_sb[pb : pb + 64, :], prep[0:HD, :])
                nc.vector.tensor_tensor(
                    y_norm_qs[qsi][pb : pb + 64, mt, :],
                    y_norm_qs[qsi][pb : pb + 64, mt, :],
                    prep_sb[pb : pb + 64, :],
                    MUL,
                )

            def section_scores(qs, h, defer):
                """Emit preload+score MMs and exp for section (qs, h).
                If defer, skip AV (return the es list); else pipeline AV."""
                mt = h // 2
                q0 = 1024 * qs
                n_live = 8 * (qs + 1)
                live_half = [min(4 * (2 * qs + j + 1), 16) for j in (0, 1)]
                es_list = []
                yT = None
                if not defer:
                    yT = psy_pool[0].tile([HD + 1, 1024], F32, tag="yT")
                pending = None
                for ki in range(n_live):
                    const_blk = 128 * ki <= q0 - 240
                    halves = [j for j in (0, 1) if ki < live_half[j]]
                    sp = psc.tile([128, 1024], F32, tag="score")
                    for j in halves:
                        if not const_blk:
                            x0 = 2048 - 128 * ki + q0 + 512 * j
                            nc.tensor.matmul(
                                sp[:, 512 * j : 512 * j + 512],
                                ident[:],
                                bands[h][:, x0 : x0 + 512],
                                start=True,
                                stop=False,
                            )
                        nc.tensor.matmul(
                            sp[:, 512 * j : 512 * j + 512],
                            kTt[mt][:, 128 * ki : 128 * ki + 128],
                            qTz[h][:, q0 + 512 * j : q0 + 512 * j + 512],
                            start=const_blk,
                            stop=True,
                        )
                    es = pes.tile([128, 1024], BF, tag="es")
                    bias = c31[:, h : h + 1] if const_blk else 0.0
                    if len(halves) == 2:
                        nc.scalar.activation(es[:], sp[:], Exp, bias=bias)
                    else:
                        j = halves[0]
                        nc.scalar.activation(
                            es[:, 512 * j : 512 * j + 512],
                            sp[:, 512 * j : 512 * j + 512],
                            Exp,
                            bias=bias,
                        )
                    es_list.append((es, halves, ki))
                    if not defer:
                        if pending is not None:
                            _av_one(yT, h, pending, live_half)
                        pending = (es, halves, ki)
                if not defer:
                    _av_one(yT, h, pending, live_half)
                    _finish_section(yT, qs, h)
                return es_list, live_half

            def _av_one(yT, h, item, live_half):
                es, halves, ki = item
                for j in halves:
                    nc.tensor.matmul(
                        yT[:, 512 * j : 512 * j + 512],
                        vxg[ki // 4][:, ki % 4, h, :],
                        es[:, 512 * j : 512 * j + 512],
                        start=(ki == 0),
                        stop=(ki == live_half[j] - 1),
                    )

            def section_av(qs, h, es_list, live_half):
                yT = psy_pool[0].tile([HD + 1, 1024], F32, tag="yT")
                for item in es_list:
                    _av_one(yT, h, item, live_half)
                _finish_section(yT, qs, h)

            def _finish_section(yT, qs, h):
                # evacuate yT (unnormalized) into its y_norm slot; the
                # denominator row goes out via ACT so DVE+ACT overlap and
                # the psy buffer frees fast.  The reciprocal chain's
                # replication+multiply for the PREVIOUS section is emitted
                # now (its rrow is long ready), so the PE never stalls on
                # the recip chain.
                mt = h // 2
                pb = 64 * (h % 2)
                nc.vector.tensor_copy(
                    y_norm_qs[qs][pb : pb + 64, mt, :], yT[0:HD, :]
                )
                dcp = pmisc.tile([1, 1024], F32, tag="dcp")
                nc.scalar.activation(dcp[:], yT[HD : HD + 1, :], Copy)
                dT = pmisc.tile([128, 8], F32, tag="dT")
                nc.sync.dma_start(dT[:], dcp[:])
                rT = pmisc.tile([128, 8], F32, tag="rT")
                nc.vector.reciprocal(rT[:], dT[:])
                rrow = pmisc.tile([1, 1024], F32, tag="rrow")
                nc.sync.dma_start(rrow[:], rT[:])
                if pending_norm[0] is not None:
                    _emit_norm(pending_norm[0])
                pending_norm[0] = (rrow, pb, mt, qs)

            def outproj(qs):
                for sl in range(2):
                    qi = 2 * qs + sl
                    for n in range(8):
                        po = pso_pool[0].tile([128, 512], F32, tag="out")
                        for c in range(2):
                            nc.tensor.matmul(
                                po[:],
                                wo[:, c, 128 * n : 128 * n + 128],
                                y_norm_qs[qs][:, c, 512 * sl : 512 * sl + 512],
                                start=(c == 0),
                                stop=(c == 1),
                            )
                        ost = pmisc.tile([128, 512], BF, tag="ost")
                        nc.vector.tensor_copy(ost[:], po[:])
                        nc.sync.dma_start(
                            outT[128 * n : 128 * n + 128, 512 * qi : 512 * qi + 512],
                            ost[:],
                        )

            # ---------------- q/k projections (m-tile major) ----------------
            psy_pool = [None]
            pso_pool = [None]

            def proj_phase(w_sb, stg_list, b_sb, is_q, m, pool):
                psums = [
                    pool.tile([128, 512], F32, tag="qk", name=f"p{m}{n}")
                    for n in range(4)
                ]
                for kc in range(8):
                    for n in range(4):
                        nc.tensor.matmul(
                            psums[n][:],
                            w_sb[:, kc, 128 * m : 128 * m + 128],
                            stg_list[kc][:, 512 * n : 512 * n + 512],
                            start=(kc == 0),
                            stop=(kc == 7),
                        )
                for n in range(4):
                    if is_q:
                        for sub in range(2):
                            pb = 64 * sub
                            nc.vector.tensor_scalar_add(
                                qTz[2 * m + sub][
                                    pb : pb + 64, 512 * n : 512 * n + 512
                                ],
                                psums[n][pb : pb + 64, :],
                                b_sb[pb : pb + 64, m : m + 1],
                            )
                    else:
                        nc.vector.tensor_scalar_add(
                            kTt[m][:, 512 * n : 512 * n + 512],
                            psums[n][:],
                            b_sb[:, m : m + 1],
                        )

            with tc.tile_pool(name="ppsum", bufs=4, space="PSUM") as pps:
                proj_phase(wq, stq, bq, True, 0, pps)
                proj_phase(wk, stk, bk, False, 0, pps)
                # heads 0/1 scores+exp start while m-tile 1 projects
                es_h0, lh_h0 = section_scores(0, 0, defer=True)
                es_h1, lh_h1 = section_scores(0, 1, defer=True)
                proj_phase(wq, stq, bq, True, 1, pps)
                proj_phase(wk, stk, bk, False, 1, pps)

            with tc.tile_pool(name="ypsum", bufs=1, space="PSUM") as psy:
                psy_pool[0] = psy
                # ---------------- v projection ----------------
                with tc.tile_pool(name="vpsum", bufs=2, space="PSUM") as psv:
                    for grp in range(2):
                        for i in range(8):
                            li = grp * 8 + i
                            pv = psv.tile([128, MPC], F32, tag="v")
                            for kc in range(8):
                                nc.tensor.matmul(
                                    pv[:],
                                    stv[kc][:, 128 * li : 128 * li + 128],
                                    wv[:, kc, :],
                                    start=(kc == 0),
                                    stop=(kc == 7),
                                )
                            nc.vector.tensor_copy(
                                vxg[li // 4][:, li % 4, :, 0:HD],
                                pv[:].rearrange("p (h d) -> p h d", h=HPC),
                            )

                # deferred AVs for heads 0/1, then the rest of qs0
                section_av(0, 0, es_h0, lh_h0)
                section_av(0, 1, es_h1, lh_h1)
                section_scores(0, 2, defer=False)
                section_scores(0, 3, defer=False)
                # first qs1 section hides the qs0 out-projection's deps
                section_scores(1, 0, defer=False)
                with tc.tile_pool(name="opsum", bufs=2, space="PSUM") as pso:
                    pso_pool[0] = pso
                    outproj(0)
                    section_scores(1, 1, defer=False)
                    section_scores(1, 2, defer=False)
                    section_scores(1, 3, defer=False)
                    _emit_norm(pending_norm[0])
                    pending_norm[0] = None
                    outproj(1)

    nc.finalize()
    return nc


def _host_tables(rel_emb: np.ndarray):
    """Per-core-group band tables; rel_emb is [NUM_BUCKETS, H]."""
    d = np.arange(4095)
    rp = d - 2047  # key - query
    buckets = _bucket(rp)
    bands = []
    c31s = []
    for h in range(H):
        vals = rel_emb[buckets, h].astype(np.float32)
        vals = np.where(rp > 0, np.float32(NEG), vals)  # causal mask
        band_pad = np.full(4223, NEG, np.float32)
        band_pad[:4095] = vals
        # BS[r, x] = band_pad[4095 + r - x]
        idx = 4095 + np.arange(128)[:, None] - np.arange(4096)[None, :]
        bands.append(band_pad[idx].astype(BF_NP))
        c31s.append(np.float32(rel_emb[31, h]))
    return bands, c31s


def _numpy_ref(query, key, value, attn_mask, key_padding_mask,
               Wq, bq, Wk, bk, Wv, bv, Wo, bo, rel_emb):
    """Exact numpy fallback for unexpected mask patterns."""
    q = (query @ Wq.T + bq).reshape(B, L, H, HD).transpose(0, 2, 1, 3)
    k = (key @ Wk.T + bk).reshape(B, L, H, HD).transpose(0, 2, 1, 3)
    v = (value @ Wv.T + bv).reshape(B, L, H, HD).transpose(0, 2, 1, 3)
    scores = np.einsum("bhqd,bhkd->bhqk", q, k) / math.sqrt(HD)
    rp = np.arange(L, dtype=np.int64)[None, :] - np.arange(L, dtype=np.int64)[:, None]
    rel = rel_emb[_bucket(rp)].transpose(2, 0, 1)
    scores = scores + rel[None]
    scores = np.where(attn_mask[None, None], scores, -np.inf)
    scores = np.where(key_padding_mask[:, None, None, :], scores, -np.inf)
    scores = scores - scores.max(-1, keepdims=True)
    e = np.exp(scores)
    attn = e / e.sum(-1, keepdims=True)
    out = np.einsum("bhqk,bhkd->bhqd", attn, v)
    out = out.transpose(0, 2, 1, 3).reshape(B, L, D)
    return (out @ Wo.T + bo).astype(np.float32)


def kernel(**inputs) -> np.ndarray:
    global _cached, last_results
    inp = {k: np.asarray(v) for k, v in inputs.items()}
    query, key, value = inp["query"], inp["key"], inp["value"]
    attn_mask, kpm = inp["attn_mask"], inp["key_padding_mask"]
    Wq, bq, Wk, bk = inp["Wq"], inp["bq"], inp["Wk"], inp["bk"]
    Wv, bv, Wo, bo = inp["Wv"], inp["bv"], inp["Wo"], inp["bo"]
    rel_emb = inp["rel_emb"]

    causal = np.array_equal(attn_mask, np.tril(np.ones((L, L), bool)))
    if not (causal and kpm.all()):
        return _numpy_ref(**inp)

    if _cached is None:
        _cached = _build()
    nc = _cached

    bands, c31s = _host_tables(rel_emb)
    ident = np.eye(128, dtype=BF_NP)

    def _rearr_w(w_slice):  # [MPC, D] row-major weights -> [128, 8, MPC]
        arr = np.ascontiguousarray(w_slice.T)  # [D, MPC]
        return arr.reshape(8, 128, MPC).transpose(1, 0, 2).astype(BF_NP)

    in_maps = []
    for c in range(N_CORES):
        b, hg = c // HPC, c % HPC
        rows = slice(MPC * hg, MPC * hg + MPC)
        heads = range(HPC * hg, HPC * hg + HPC)
        wo_c = np.ascontiguousarray(Wo[:, rows].T)  # [MPC, D]
        in_maps.append({
            "qT_in": np.ascontiguousarray(query[b].T).astype(BF_NP),
            "kT_in": np.ascontiguousarray(key[b].T).astype(BF_NP),
            "vT_in": np.ascontiguousarray(value[b].T).astype(BF_NP),
            "wq_in": _rearr_w(Wq[rows] / math.sqrt(HD)),
            "wk_in": _rearr_w(Wk[rows]),
            "wv_in": _rearr_w(Wv[rows]),
            "wo_in": wo_c.reshape(2, 128, D).transpose(1, 0, 2).astype(BF_NP),
            "bq_in": np.ascontiguousarray(
                (bq[rows] / math.sqrt(HD)).reshape(2, 128).T.astype(np.float32)
            ),
            "bk_in": np.ascontiguousarray(
                bk[rows].reshape(2, 128).T.astype(np.float32)
            ),
            "band_in": np.stack([bands[h] for h in heads]),
            "c31_in": np.tile(
                np.array([c31s[h] for h in heads], np.float32), (128, 1)
            ),
            "id_in": ident,
        })

    res = run_bass_kernel_spmd(nc, in_maps, list(range(N_CORES)))
    last_results = res

    bo_eff = (
        bo.astype(np.float64) + bv.astype(np.float64) @ Wo.T.astype(np.float64)
    )
    out = np.empty((B, L, D), np.float32)
    for b in range(B):
        acc = np.zeros((D, L), np.float64)
        for hg in range(HPC):
            acc += res.results[b * HPC + hg]["outT"].astype(np.float64)
        out[b] = (acc.T + bo_eff[None, :]).astype(np.float32)
    return out


# revision 19
# speedup vs baseline: 1.1783x; 1.1783x over previous
"""Bass/Trainium2 kernel for nn_MultiHeadAttention (T5-style rel-bias causal MHA).

Sharding: 8 cores = 2 batches x 4 head-groups (4 heads of 64 dims each).
Each core: projects q/k/v for its 256 proj rows (bf16 operands, fp32 PSUM),
runs causal attention with the T5 relative bias folded in as either a PSUM
band preload (near-diagonal blocks) or a constant exp-bias
(bucket-31-saturated blocks), and computes a partial out-projection.
Host sums the 4 bf16 partials per batch.

v2 layout: bf16 operands everywhere (halves HBM traffic vs f32r), m-tile
ordered q/k projection so attention sections for heads 0/1 start early
(their AV deferred until v-proj lands), qs-outer section order with the
qs0 out-projection overlapped under qs1 attention, and the scalar engine
kept free of DMA issue so it does nothing but exp.
"""
import math
import sys

sys.path.insert(0, "/opt/trn_rl_repo")

import numpy as np
import ml_dtypes

from concourse import bacc
import concourse.mybir as mybir
import concourse.tile as tile
from concourse.bass_utils import run_bass_kernel_spmd

F32 = mybir.dt.float32
F32R = mybir.dt.float32r
BF = mybir.dt.bfloat16
Exp = mybir.ActivationFunctionType.Exp
Copy = mybir.ActivationFunctionType.Copy
MUL = mybir.AluOpType.mult

B, L, D = 2, 2048, 1024
H, HD = 16, 64
NUM_BUCKETS, MAX_DISTANCE = 32, 128
HPC = 4  # heads per core
MPC = HPC * HD  # 256 proj rows per core
N_CORES = 8
NEG = -60.0  # additive mask value (exp(-60+s) == 0 in practice)
BF_NP = ml_dtypes.bfloat16

last_results = None  # BassKernelResults of the most recent run (for profiling)
_cached = None


def _bucket(rp: np.ndarray) -> np.ndarray:
    """T5 relative position bucket, mirrors the reference exactly."""
    sign = (rp > 0).astype(np.int32)
    n = np.abs(rp)
    max_exact = NUM_BUCKETS // 2
    n_safe = np.maximum(n, 1).astype(np.float32)
    vil = max_exact + (
        np.log(n_safe / max_exact)
        / math.log(MAX_DISTANCE / max_exact)
        * (NUM_BUCKETS - max_exact)
    ).astype(np.int32)
    vil = np.minimum(vil, NUM_BUCKETS - 1)
    buckets = np.where(n < max_exact, n, vil) + sign * max_exact
    return np.clip(buckets, 0, NUM_BUCKETS - 1)


def _build():
    nc = bacc.Bacc(trn_type="TRN2")

    qT_in = nc.dram_tensor("qT_in", [D, L], BF, kind="ExternalInput")
    kT_in = nc.dram_tensor("kT_in", [D, L], BF, kind="ExternalInput")
    vT_in = nc.dram_tensor("vT_in", [D, L], BF, kind="ExternalInput")
    wq_in = nc.dram_tensor("wq_in", [128, 8, MPC], BF, kind="ExternalInput")
    wk_in = nc.dram_tensor("wk_in", [128, 8, MPC], BF, kind="ExternalInput")
    wv_in = nc.dram_tensor("wv_in", [128, 8, MPC], BF, kind="ExternalInput")
    wo_in = nc.dram_tensor("wo_in", [128, 2, D], BF, kind="ExternalInput")
    bq_in = nc.dram_tensor("bq_in", [128, 2], F32, kind="ExternalInput")
    bk_in = nc.dram_tensor("bk_in", [128, 2], F32, kind="ExternalInput")
    # band columns 0..1151 are never addressed (x0 >= 1152): trimmed
    band_in = nc.dram_tensor("band_in", [HPC, 128, 2944], BF, kind="ExternalInput")
    c31_in = nc.dram_tensor("c31_in", [128, HPC], F32, kind="ExternalInput")
    id_in = nc.dram_tensor("id_in", [128, 128], BF, kind="ExternalInput")
    # [p, n, l] layout: row (128n + p) of the [D, L] partial lives at [p, n, l]
    outT = nc.dram_tensor("outT", [128, 8, L], BF, kind="ExternalOutput")

    with tile.TileContext(nc) as tc:
        with (
            tc.tile_pool(name="res", bufs=1) as pr,
            tc.tile_pool(name="qkv", bufs=1) as pqkv,
            tc.tile_pool(name="stg", bufs=16) as pstg,
            tc.tile_pool(name="es", bufs=16) as pes,
            tc.tile_pool(name="misc", bufs=2) as pmisc,
            tc.tile_pool(name="spsum", bufs=2, space="PSUM") as psc,
        ):
            # ---- input DMAs: qT chunks on sync, everything else gpsimd ----
            # scalar's HWDGE ring is idle until the first exp (~30us), so
            # odd chunks ride it for 2x input stream rate during proj.
            stq = []
            for kc in range(8):
                t = pstg.tile([128, L], BF, tag="stage", name=f"sq{kc}")
                eng = nc.sync if kc % 2 == 0 else nc.scalar
                eng.dma_start(t[:], qT_in[128 * kc : 128 * kc + 128, :])
                stq.append(t)
            stk = []
            for kc in range(8):
                t = pstg.tile([128, L], BF, tag="stage", name=f"sk{kc}")
                eng = nc.sync if kc % 2 == 0 else nc.scalar
                eng.dma_start(t[:], kT_in[128 * kc : 128 * kc + 128, :])
                stk.append(t)
            wq = pr.tile([128, 8, MPC], BF)
            nc.gpsimd.dma_start(wq[:], wq_in[:])
            wk = pr.tile([128, 8, MPC], BF)
            nc.gpsimd.dma_start(wk[:], wk_in[:])
            bq = pr.tile([128, 2], F32)
            nc.gpsimd.dma_start(bq[:], bq_in[:])
            bk = pr.tile([128, 2], F32)
            nc.gpsimd.dma_start(bk[:], bk_in[:])
            c31 = pr.tile([128, HPC], F32)
            nc.gpsimd.dma_start(c31[:], c31_in[:])
            ident = pr.tile([128, 128], BF)
            nc.gpsimd.dma_start(ident[:], id_in[:])
            bands = []
            for h in range(HPC):
                t = pr.tile([128, 2944], BF, name=f"band{h}")
                nc.gpsimd.dma_start(t[:], band_in[h])
                bands.append(t)
            # v input staged behind q (reuses q's stage bufs), issued on sync
            stv = []
            for kc in range(8):
                t = pstg.tile([128, L], BF, tag="stage", name=f"sv{kc}")
                nc.sync.dma_start(t[:], vT_in[128 * kc : 128 * kc + 128, :])
                stv.append(t)
            wv = pr.tile([128, 8, MPC], BF)
            nc.gpsimd.dma_start(wv[:], wv_in[:])
            wo = pr.tile([128, 2, D], BF)
            nc.gpsimd.dma_start(wo[:], wo_in[:])

            # warm the ACT exp table early, off the critical path
            warm = pr.tile([1, 2], F32)
            nc.vector.memset(warm[:], 0.0)
            nc.scalar.activation(warm[:], warm[:], Exp)
            ones_v = pr.tile([1, HD], BF)
            nc.vector.memset(ones_v[:], 1.0)

            qTz = []
            for hh in range(HPC):
                t = pqkv.tile([128, L], BF, name=f"qtz{hh}")
                nc.vector.memset(t[:].bitcast(F32), 0)
                qTz.append(t)
            kTt = [pqkv.tile([128, L], BF, name=f"kt{mm}") for mm in range(2)]
            vxg = []
            for g in range(4):
                t = pqkv.tile([128, 4, HPC, HD + 1], BF, name=f"vx{g}")
                nc.vector.memset(t[:, :, :, HD], 1.0)
                vxg.append(t)
            y_norm_qs = [
                pqkv.tile([128, 2, 1024], BF, name=f"yn{qq}") for qq in range(2)
            ]

            # ---------------- attention section machinery ----------------
            pending_norm = [None]

            def _emit_norm(item):
                # PE-side replication of the reciprocal row + in-place
                # multiply; prep rides the score-psum ring.
                rrow, pb, mt, qsi = item
                prep = psc.tile([128, 1024], F32, tag="score", name="prep")
                nc.tensor.matmul(
                    prep[0:HD, :512], ones_v[:], rrow[:, :512],
                    start=True, stop=True,
                )
                nc.tensor.matmul(
                    prep[0:HD, 512:], ones_v[:], rrow[:, 512:],
                    start=True, stop=True,
                )
                prep_sb = pmisc.tile([128, 1024], BF, tag="prep")
                nc.vector.tensor_copy(prep_sb[pb : pb + 64, :], prep[0:HD, :])
                nc.vector.tensor_tensor(
                    y_norm_qs[qsi][pb : pb + 64, mt, :],
                    y_norm_qs[qsi][pb : pb + 64, mt, :],
                    prep_sb[pb : pb + 64, :],
                    MUL,
                )

            def section_scores(qs, h, defer):
                """Emit preload+score MMs and exp for section (qs, h).
                If defer, skip AV (return the es list); else pipeline AV."""
                mt = h // 2
                q0 = 1024 * qs
                n_live = 8 * (qs + 1)
                live_half = [min(4 * (2 * qs + j + 1), 16) for j in (0, 1)]
                es_list = []
                yT = None
                if not defer:
                    yT = psy_pool[0].tile([HD + 1, 1024], F32, tag="yT")
                pending = None
                for ki in range(n_live):
                    const_blk = 128 * ki <= q0 - 240
                    halves = [j for j in (0, 1) if ki < live_half[j]]
                    sp = psc.tile([128, 1024], F32, tag="score")
                    for j in halves:
                        if not const_blk:
                            x0 = 2048 - 128 * ki + q0 + 512 * j - 1152
                            nc.tensor.matmul(
                                sp[:, 512 * j : 512 * j + 512],
                                ident[:],
                                bands[h][:, x0 : x0 + 512],
                                start=True,
                                stop=False,
                            )
                        nc.tensor.matmul(
                            sp[:, 512 * j : 512 * j + 512],
                            kTt[mt][:, 128 * ki : 128 * ki + 128],
                            qTz[h][:, q0 + 512 * j : q0 + 512 * j + 512],
                            start=const_blk,
                            stop=True,
                        )
                    es = pes.tile([128, 1024], BF, tag="es")
                    bias = c31[:, h : h + 1] if const_blk else 0.0
                    if len(halves) == 2:
                        nc.scalar.activation(es[:], sp[:], Exp, bias=bias)
                    else:
                        j = halves[0]
                        nc.scalar.activation(
                            es[:, 512 * j : 512 * j + 512],
                            sp[:, 512 * j : 512 * j + 512],
                            Exp,
                            bias=bias,
                        )
                    es_list.append((es, halves, ki))
                    if not defer:
                        if pending is not None:
                            _av_one(yT, h, pending, live_half)
                        pending = (es, halves, ki)
                if not defer:
                    _av_one(yT, h, pending, live_half)
                    _finish_section(yT, qs, h)
                return es_list, live_half

            def _av_one(yT, h, item, live_half):
                es, halves, ki = item
                for j in halves:
                    nc.tensor.matmul(
                        yT[:, 512 * j : 512 * j + 512],
                        vxg[ki // 4][:, ki % 4, h, :],
                        es[:, 512 * j : 512 * j + 512],
                        start=(ki == 0),
                        stop=(ki == live_half[j] - 1),
                    )

            def section_av(qs, h, es_list, live_half):
                yT = psy_pool[0].tile([HD + 1, 1024], F32, tag="yT")
                for item in es_list:
                    _av_one(yT, h, item, live_half)
                _finish_section(yT, qs, h)

            def _finish_section(yT, qs, h):
                # evacuate yT (unnormalized) into its y_norm slot; the
                # denominator row goes out via ACT so DVE+ACT overlap and
                # the psy buffer frees fast.  The reciprocal chain's
                # replication+multiply for the PREVIOUS section is emitted
                # now (its rrow is long ready), so the PE never stalls on
                # the recip chain.
                mt = h // 2
                pb = 64 * (h % 2)
                nc.vector.tensor_copy(
                    y_norm_qs[qs][pb : pb + 64, mt, :], yT[0:HD, :]
                )
                dcp = pmisc.tile([1, 1024], F32, tag="dcp")
                nc.scalar.activation(dcp[:], yT[HD : HD + 1, :], Copy)
                rrow = pmisc.tile([1, 1024], BF, tag="rrow")
                with nc.allow_low_precision(reason="softmax recip bf16"):
                    nc.vector.reciprocal(rrow[:], dcp[:])
                if pending_norm[0] is not None:
                    _emit_norm(pending_norm[0])
                pending_norm[0] = (rrow, pb, mt, qs)

            def outproj(qs):
                # stage the whole 512-query slab in SBUF, write it with ONE
                # DMA: avoids the ~2us per-DMA completion cost serializing
                # the PSUM ring at the tail.
                for sl in range(2):
                    qi = 2 * qs + sl
                    ost = pmisc.tile([128, 8, 512], BF, tag="ost")
                    for n in range(8):
                        po = pso_pool[0].tile([128, 512], F32, tag="out")
                        for c in range(2):
                            nc.tensor.matmul(
                                po[:],
                                wo[:, c, 128 * n : 128 * n + 128],
                                y_norm_qs[qs][:, c, 512 * sl : 512 * sl + 512],
                                start=(c == 0),
                                stop=(c == 1),
                            )
                        nc.vector.tensor_copy(ost[:, n, :], po[:])
                    nc.sync.dma_start(
                        outT[:, :, 512 * qi : 512 * qi + 512], ost[:]
                    )

            # ---------------- q/k projections (m-tile major) ----------------
            psy_pool = [None]
            pso_pool = [None]

            def proj_phase(w_sb, stg_list, b_sb, is_q, m, pool):
                psums = [
                    pool.tile([128, 512], F32, tag="qk", name=f"p{m}{n}")
                    for n in range(4)
                ]
                for kc in range(8):
                    for n in range(4):
                        nc.tensor.matmul(
                            psums[n][:],
                            w_sb[:, kc, 128 * m : 128 * m + 128],
                            stg_list[kc][:, 512 * n : 512 * n + 512],
                            start=(kc == 0),
                            stop=(kc == 7),
                        )
                for n in range(4):
                    if is_q:
                        for sub in range(2):
                            pb = 64 * sub
                            nc.vector.tensor_scalar_add(
                                qTz[2 * m + sub][
                                    pb : pb + 64, 512 * n : 512 * n + 512
                                ],
                                psums[n][pb : pb + 64, :],
                                b_sb[pb : pb + 64, m : m + 1],
                            )
                    else:
                        nc.vector.tensor_scalar_add(
                            kTt[m][:, 512 * n : 512 * n + 512],
                            psums[n][:],
                            b_sb[:, m : m + 1],
                        )

            with tc.tile_pool(name="ppsum", bufs=4, space="PSUM") as pps:
                proj_phase(wq, stq, bq, True, 0, pps)
                proj_phase(wk, stk, bk, False, 0, pps)
                # heads 0/1 scores+exp start while m-tile 1 projects
                es_h0, lh_h0 = section_scores(0, 0, defer=True)
                es_h1, lh_h1 = section_scores(0, 1, defer=True)
                proj_phase(wq, stq, bq, True, 1, pps)
                proj_phase(wk, stk, bk, False, 1, pps)

            with tc.tile_pool(name="ypsum", bufs=1, space="PSUM") as psy:
                psy_pool[0] = psy
                # ---------------- v projection ----------------
                with tc.tile_pool(name="vpsum", bufs=2, space="PSUM") as psv:
                    for grp in range(2):
                        for i in range(8):
                            li = grp * 8 + i
                            pv = psv.tile([128, MPC], F32, tag="v")
                            for kc in range(8):
                                nc.tensor.matmul(
                                    pv[:],
                                    stv[kc][:, 128 * li : 128 * li + 128],
                                    wv[:, kc, :],
                                    start=(kc == 0),
                                    stop=(kc == 7),
                                )
                            nc.vector.tensor_copy(
                                vxg[li // 4][:, li % 4, :, 0:HD],
                                pv[:].rearrange("p (h d) -> p h d", h=HPC),
                            )

                # deferred AVs for heads 0/1, then the rest of qs0
                section_av(0, 0, es_h0, lh_h0)
                section_av(0, 1, es_h1, lh_h1)
                section_scores(0, 2, defer=False)
                section_scores(0, 3, defer=False)
                # first qs1 section hides the qs0 out-projection's deps
                section_scores(1, 0, defer=False)
                with tc.tile_pool(name="opsum", bufs=2, space="PSUM") as pso:
                    pso_pool[0] = pso
                    outproj(0)
                    section_scores(1, 1, defer=False)
                    section_scores(1, 2, defer=False)
                    section_scores(1, 3, defer=False)
                    _emit_norm(pending_norm[0])
                    pending_norm[0] = None
                    outproj(1)

    nc.finalize()
    return nc


def _host_tables(rel_emb: np.ndarray):
    """Per-core-group band tables; rel_emb is [NUM_BUCKETS, H]."""
    d = np.arange(4095)
    rp = d - 2047  # key - query
    buckets = _bucket(rp)
    bands = []
    c31s = []
    for h in range(H):
        vals = rel_emb[buckets, h].astype(np.float32)
        vals = np.where(rp > 0, np.float32(NEG), vals)  # causal mask
        band_pad = np.full(4223, NEG, np.float32)
        band_pad[:4095] = vals
        # BS[r, x] = band_pad[4095 + r - (x + 1152)]  (cols < 1152 unused)
        idx = 4095 + np.arange(128)[:, None] - np.arange(1152, 4096)[None, :]
        bands.append(band_pad[idx].astype(BF_NP))
        c31s.append(np.float32(rel_emb[31, h]))
    return bands, c31s


def _numpy_ref(query, key, value, attn_mask, key_padding_mask,
               Wq, bq, Wk, bk, Wv, bv, Wo, bo, rel_emb):
    """Exact numpy fallback for unexpected mask patterns."""
    q = (query @ Wq.T + bq).reshape(B, L, H, HD).transpose(0, 2, 1, 3)
    k = (key @ Wk.T + bk).reshape(B, L, H, HD).transpose(0, 2, 1, 3)
    v = (value @ Wv.T + bv).reshape(B, L, H, HD).transpose(0, 2, 1, 3)
    scores = np.einsum("bhqd,bhkd->bhqk", q, k) / math.sqrt(HD)
    rp = np.arange(L, dtype=np.int64)[None, :] - np.arange(L, dtype=np.int64)[:, None]
    rel = rel_emb[_bucket(rp)].transpose(2, 0, 1)
    scores = scores + rel[None]
    scores = np.where(attn_mask[None, None], scores, -np.inf)
    scores = np.where(key_padding_mask[:, None, None, :], scores, -np.inf)
    scores = scores - scores.max(-1, keepdims=True)
    e = np.exp(scores)
    attn = e / e.sum(-1, keepdims=True)
    out = np.einsum("bhqk,bhkd->bhqd", attn, v)
    out = out.transpose(0, 2, 1, 3).reshape(B, L, D)
    return (out @ Wo.T + bo).astype(np.float32)


def kernel(**inputs) -> np.ndarray:
    global _cached, last_results
    inp = {k: np.asarray(v) for k, v in inputs.items()}
    query, key, value = inp["query"], inp["key"], inp["value"]
    attn_mask, kpm = inp["attn_mask"], inp["key_padding_mask"]
    Wq, bq, Wk, bk = inp["Wq"], inp["bq"], inp["Wk"], inp["bk"]
    Wv, bv, Wo, bo = inp["Wv"], inp["bv"], inp["Wo"], inp["bo"]
    rel_emb = inp["rel_emb"]

    causal = np.array_equal(attn_mask, np.tril(np.ones((L, L), bool)))
    if not (causal and kpm.all()):
        return _numpy_ref(**inp)

    if _cached is None:
        _cached = _build()
    nc = _cached

    bands, c31s = _host_tables(rel_emb)
    ident = np.eye(128, dtype=BF_NP)

    def _rearr_w(w_slice):  # [MPC, D] row-major weights -> [128, 8, MPC]
        arr = np.ascontiguousarray(w_slice.T)  # [D, MPC]
        return arr.reshape(8, 128, MPC).transpose(1, 0, 2).astype(BF_NP)

    in_maps = []
    for c in range(N_CORES):
        b, hg = c // HPC, c % HPC
        rows = slice(MPC * hg, MPC * hg + MPC)
        heads = range(HPC * hg, HPC * hg + HPC)
        wo_c = np.ascontiguousarray(Wo[:, rows].T)  # [MPC, D]
        in_maps.append({
            "qT_in": np.ascontiguousarray(query[b].T).astype(BF_NP),
            "kT_in": np.ascontiguousarray(key[b].T).astype(BF_NP),
            "vT_in": np.ascontiguousarray(value[b].T).astype(BF_NP),
            "wq_in": _rearr_w(Wq[rows] / math.sqrt(HD)),
            "wk_in": _rearr_w(Wk[rows]),
            "wv_in": _rearr_w(Wv[rows]),
            "wo_in": wo_c.reshape(2, 128, D).transpose(1, 0, 2).astype(BF_NP),
            "bq_in": np.ascontiguousarray(
                (bq[rows] / math.sqrt(HD)).reshape(2, 128).T.astype(np.float32)
            ),
            "bk_in": np.ascontiguousarray(
                bk[rows].reshape(2, 128).T.astype(np.float32)
            ),
            "band_in": np.stack([bands[h] for h in heads]),
            "c31_in": np.tile(
                np.array([c31s[h] for h in heads], np.float32), (128, 1)
            ),
            "id_in": ident,
        })

    res = run_bass_kernel_spmd(nc, in_maps, list(range(N_CORES)))
    last_results = res

    bo_eff = (
        bo.astype(np.float64) + bv.astype(np.float64) @ Wo.T.astype(np.float64)
    )
    out = np.empty((B, L, D), np.float32)
    for b in range(B):
        acc = np.zeros((D, L), np.float64)
        for hg in range(HPC):
            part = res.results[b * HPC + hg]["outT"].astype(np.float64)
            acc += part.transpose(1, 0, 2).reshape(D, L)
        out[b] = (acc.T + bo_eff[None, :]).astype(np.float32)
    return out


# revision 20
# speedup vs baseline: 1.2566x; 1.0665x over previous
"""Bass/Trainium2 kernel for nn_MultiHeadAttention (T5-style rel-bias causal MHA).

Sharding: 8 cores = 2 batches x 4 head-groups (4 heads of 64 dims each).
Each core: projects q/k/v for its 256 proj rows (bf16 operands, fp32 PSUM),
runs causal attention with the T5 relative bias folded in as either a PSUM
band preload (near-diagonal blocks) or a constant exp-bias
(bucket-31-saturated blocks), and computes a partial out-projection.
Host sums the 4 bf16 partials per batch.

v2 layout: bf16 operands everywhere (halves HBM traffic vs f32r), m-tile
ordered q/k projection so attention sections for heads 0/1 start early
(their AV deferred until v-proj lands), qs-outer section order with the
qs0 out-projection overlapped under qs1 attention, and the scalar engine
kept free of DMA issue so it does nothing but exp.
"""
import math
import sys

sys.path.insert(0, "/opt/trn_rl_repo")

import numpy as np
import ml_dtypes

from concourse import bacc
import concourse.mybir as mybir
import concourse.tile as tile
from concourse.bass_utils import run_bass_kernel_spmd

F32 = mybir.dt.float32
F32R = mybir.dt.float32r
BF = mybir.dt.bfloat16
Exp = mybir.ActivationFunctionType.Exp
Copy = mybir.ActivationFunctionType.Copy
MUL = mybir.AluOpType.mult

B, L, D = 2, 2048, 1024
H, HD = 16, 64
NUM_BUCKETS, MAX_DISTANCE = 32, 128
HPC = 4  # heads per core
MPC = HPC * HD  # 256 proj rows per core
N_CORES = 8
NEG = -60.0  # additive mask value (exp(-60+s) == 0 in practice)
BF_NP = ml_dtypes.bfloat16

last_results = None  # BassKernelResults of the most recent run (for profiling)
_cached = None


def _bucket(rp: np.ndarray) -> np.ndarray:
    """T5 relative position bucket, mirrors the reference exactly."""
    sign = (rp > 0).astype(np.int32)
    n = np.abs(rp)
    max_exact = NUM_BUCKETS // 2
    n_safe = np.maximum(n, 1).astype(np.float32)
    vil = max_exact + (
        np.log(n_safe / max_exact)
        / math.log(MAX_DISTANCE / max_exact)
        * (NUM_BUCKETS - max_exact)
    ).astype(np.int32)
    vil = np.minimum(vil, NUM_BUCKETS - 1)
    buckets = np.where(n < max_exact, n, vil) + sign * max_exact
    return np.clip(buckets, 0, NUM_BUCKETS - 1)


def _build():
    nc = bacc.Bacc(trn_type="TRN2")

    qT_in = nc.dram_tensor("qT_in", [D, L], BF, kind="ExternalInput")
    kT_in = nc.dram_tensor("kT_in", [D, L], BF, kind="ExternalInput")
    vT_in = nc.dram_tensor("vT_in", [D, L], BF, kind="ExternalInput")
    wq_in = nc.dram_tensor("wq_in", [128, 8, MPC], BF, kind="ExternalInput")
    wk_in = nc.dram_tensor("wk_in", [128, 8, MPC], BF, kind="ExternalInput")
    wv_in = nc.dram_tensor("wv_in", [128, 8, MPC], BF, kind="ExternalInput")
    wo_in = nc.dram_tensor("wo_in", [128, 2, D], BF, kind="ExternalInput")
    bq_in = nc.dram_tensor("bq_in", [128, 2], F32, kind="ExternalInput")
    bk_in = nc.dram_tensor("bk_in", [128, 2], F32, kind="ExternalInput")
    # band columns 0..1151 are never addressed (x0 >= 1152): trimmed
    band_in = nc.dram_tensor("band_in", [HPC, 128, 2944], BF, kind="ExternalInput")
    c31_in = nc.dram_tensor("c31_in", [128, HPC], F32, kind="ExternalInput")
    id_in = nc.dram_tensor("id_in", [128, 128], BF, kind="ExternalInput")
    # [p, n, l] layout: row (128n + p) of the [D, L] partial lives at [p, n, l]
    outT = nc.dram_tensor("outT", [128, 8, L], BF, kind="ExternalOutput")

    with tile.TileContext(nc) as tc:
        with (
            tc.tile_pool(name="res", bufs=1) as pr,
            tc.tile_pool(name="qkv", bufs=1) as pqkv,
            tc.tile_pool(name="stg", bufs=16) as pstg,
            tc.tile_pool(name="es", bufs=16) as pes,
            tc.tile_pool(name="misc", bufs=2) as pmisc,
            tc.tile_pool(name="spsum", bufs=2, space="PSUM") as psc,
        ):
            # ---- input DMAs: qT chunks on sync, everything else gpsimd ----
            # scalar's HWDGE ring is idle until the first exp (~30us), so
            # odd chunks ride it for 2x input stream rate during proj.
            stq = []
            for kc in range(8):
                t = pstg.tile([128, L], BF, tag="stage", name=f"sq{kc}")
                eng = nc.sync if kc % 2 == 0 else nc.scalar
                eng.dma_start(t[:], qT_in[128 * kc : 128 * kc + 128, :])
                stq.append(t)
            stk = []
            for kc in range(8):
                t = pstg.tile([128, L], BF, tag="stage", name=f"sk{kc}")
                eng = nc.sync if kc % 2 == 0 else nc.scalar
                eng.dma_start(t[:], kT_in[128 * kc : 128 * kc + 128, :])
                stk.append(t)
            wq = pr.tile([128, 8, MPC], BF)
            nc.gpsimd.dma_start(wq[:], wq_in[:])
            wk = pr.tile([128, 8, MPC], BF)
            nc.gpsimd.dma_start(wk[:], wk_in[:])
            bq = pr.tile([128, 2], F32)
            nc.gpsimd.dma_start(bq[:], bq_in[:])
            bk = pr.tile([128, 2], F32)
            nc.gpsimd.dma_start(bk[:], bk_in[:])
            c31 = pr.tile([128, HPC], F32)
            nc.gpsimd.dma_start(c31[:], c31_in[:])
            ident = pr.tile([128, 128], BF)
            nc.gpsimd.dma_start(ident[:], id_in[:])
            bands = []
            for h in range(HPC):
                t = pr.tile([128, 2944], BF, name=f"band{h}")
                nc.gpsimd.dma_start(t[:], band_in[h])
                bands.append(t)
            # v input staged behind q (reuses q's stage bufs), issued on sync
            stv = []
            for kc in range(8):
                t = pstg.tile([128, L], BF, tag="stage", name=f"sv{kc}")
                nc.sync.dma_start(t[:], vT_in[128 * kc : 128 * kc + 128, :])
                stv.append(t)
            wv = pr.tile([128, 8, MPC], BF)
            nc.gpsimd.dma_start(wv[:], wv_in[:])
            wo = pr.tile([128, 2, D], BF)
            nc.gpsimd.dma_start(wo[:], wo_in[:])

            # warm the ACT exp table early, off the critical path
            warm = pr.tile([1, 2], F32)
            nc.vector.memset(warm[:], 0.0)
            nc.scalar.activation(warm[:], warm[:], Exp)
            ones_v = pr.tile([1, HD], BF)
            nc.vector.memset(ones_v[:], 1.0)

            qTz = []
            for hh in range(HPC):
                t = pqkv.tile([128, L], BF, name=f"qtz{hh}")
                nc.vector.memset(t[:].bitcast(F32), 0)
                qTz.append(t)
            kTt = [pqkv.tile([128, L], BF, name=f"kt{mm}") for mm in range(2)]
            vxg = []
            for g in range(4):
                t = pqkv.tile([128, 4, HPC, HD + 1], BF, name=f"vx{g}")
                nc.vector.memset(t[:, :, :, HD], 1.0)
                vxg.append(t)
            y_norm_qs = [
                pqkv.tile([128, 2, 1024], BF, name=f"yn{qq}") for qq in range(2)
            ]

            # ---------------- attention section machinery ----------------
            pending_norm = [None]

            def _emit_norm(item):
                # PE-side replication of the reciprocal row + in-place
                # multiply; prep rides the score-psum ring.
                rrow, pb, mt, qsi = item
                prep = psc.tile([128, 1024], F32, tag="score", name="prep")
                nc.tensor.matmul(
                    prep[0:HD, :512], ones_v[:], rrow[:, :512],
                    start=True, stop=True,
                )
                nc.tensor.matmul(
                    prep[0:HD, 512:], ones_v[:], rrow[:, 512:],
                    start=True, stop=True,
                )
                prep_sb = pmisc.tile([128, 1024], BF, tag="prep")
                nc.vector.tensor_copy(prep_sb[pb : pb + 64, :], prep[0:HD, :])
                nc.vector.tensor_tensor(
                    y_norm_qs[qsi][pb : pb + 64, mt, :],
                    y_norm_qs[qsi][pb : pb + 64, mt, :],
                    prep_sb[pb : pb + 64, :],
                    MUL,
                )

            def section_scores(qs, h, defer):
                """Emit preload+score MMs and exp for section (qs, h).
                If defer, skip AV (return the es list); else pipeline AV."""
                mt = h // 2
                q0 = 1024 * qs
                n_live = 8 * (qs + 1)
                live_half = [min(4 * (2 * qs + j + 1), 16) for j in (0, 1)]
                es_list = []
                yT = None
                if not defer:
                    yT = psy_pool[0].tile([HD + 1, 1024], F32, tag="yT")
                pending = None
                for ki in range(n_live):
                    const_blk = 128 * ki <= q0 - 240
                    halves = [j for j in (0, 1) if ki < live_half[j]]
                    sp = psc.tile([128, 1024], F32, tag="score")
                    for j in halves:
                        if not const_blk:
                            x0 = 2048 - 128 * ki + q0 + 512 * j - 1152
                            nc.tensor.matmul(
                                sp[:, 512 * j : 512 * j + 512],
                                ident[:],
                                bands[h][:, x0 : x0 + 512],
                                start=True,
                                stop=False,
                            )
                        nc.tensor.matmul(
                            sp[:, 512 * j : 512 * j + 512],
                            kTt[mt][:, 128 * ki : 128 * ki + 128],
                            qTz[h][:, q0 + 512 * j : q0 + 512 * j + 512],
                            start=const_blk,
                            stop=True,
                        )
                    es = pes.tile([128, 1024], BF, tag="es")
                    bias = c31[:, h : h + 1] if const_blk else 0.0
                    if len(halves) == 2:
                        nc.scalar.activation(es[:], sp[:], Exp, bias=bias)
                    else:
                        j = halves[0]
                        nc.scalar.activation(
                            es[:, 512 * j : 512 * j + 512],
                            sp[:, 512 * j : 512 * j + 512],
                            Exp,
                            bias=bias,
                        )
                    es_list.append((es, halves, ki))
                    if not defer:
                        if pending is not None:
                            _av_one(yT, h, pending, live_half)
                        pending = (es, halves, ki)
                if not defer:
                    _av_one(yT, h, pending, live_half)
                    _finish_section(yT, qs, h)
                return es_list, live_half

            def _av_one(yT, h, item, live_half):
                es, halves, ki = item
                for j in halves:
                    nc.tensor.matmul(
                        yT[:, 512 * j : 512 * j + 512],
                        vxg[ki // 4][:, ki % 4, h, :],
                        es[:, 512 * j : 512 * j + 512],
                        start=(ki == 0),
                        stop=(ki == live_half[j] - 1),
                    )

            def section_av(qs, h, es_list, live_half):
                yT = psy_pool[0].tile([HD + 1, 1024], F32, tag="yT")
                for item in es_list:
                    _av_one(yT, h, item, live_half)
                _finish_section(yT, qs, h)

            def _finish_section(yT, qs, h):
                # evacuate yT (unnormalized) into its y_norm slot; the
                # denominator row goes out via ACT so DVE+ACT overlap and
                # the psy buffer frees fast.  The reciprocal chain's
                # replication+multiply for the PREVIOUS section is emitted
                # now (its rrow is long ready), so the PE never stalls on
                # the recip chain.
                mt = h // 2
                pb = 64 * (h % 2)
                nc.vector.tensor_copy(
                    y_norm_qs[qs][pb : pb + 64, mt, :], yT[0:HD, :]
                )
                dcp = pmisc.tile([1, 1024], F32, tag="dcp")
                nc.scalar.activation(dcp[:], yT[HD : HD + 1, :], Copy)
                dT = pmisc.tile([128, 8], F32, tag="dT")
                nc.sync.dma_start(dT[:], dcp[:])
                rT = pmisc.tile([128, 8], BF, tag="rT")
                with nc.allow_low_precision(reason="softmax recip bf16"):
                    nc.vector.reciprocal(rT[:], dT[:])
                rrow = pmisc.tile([1, 1024], BF, tag="rrow")
                nc.sync.dma_start(rrow[:], rT[:])
                if pending_norm[0] is not None:
                    _emit_norm(pending_norm[0])
                pending_norm[0] = (rrow, pb, mt, qs)

            def outproj(qs):
                # stage the whole 512-query slab in SBUF, write it with ONE
                # DMA: avoids the ~2us per-DMA completion cost serializing
                # the PSUM ring at the tail.
                for sl in range(2):
                    qi = 2 * qs + sl
                    ost = pmisc.tile([128, 8, 512], BF, tag="ost")
                    for n in range(8):
                        po = pso_pool[0].tile([128, 512], F32, tag="out")
                        for c in range(2):
                            nc.tensor.matmul(
                                po[:],
                                wo[:, c, 128 * n : 128 * n + 128],
                                y_norm_qs[qs][:, c, 512 * sl : 512 * sl + 512],
                                start=(c == 0),
                                stop=(c == 1),
                            )
                        nc.vector.tensor_copy(ost[:, n, :], po[:])
                    nc.sync.dma_start(
                        outT[:, :, 512 * qi : 512 * qi + 512], ost[:]
                    )

            # ---------------- q/k projections (m-tile major) ----------------
            psy_pool = [None]
            pso_pool = [None]

            def proj_phase(w_sb, stg_list, b_sb, is_q, m, pool):
                psums = [
                    pool.tile([128, 512], F32, tag="qk", name=f"p{m}{n}")
                    for n in range(4)
                ]
                for kc in range(8):
                    for n in range(4):
                        nc.tensor.matmul(
                            psums[n][:],
                            w_sb[:, kc, 128 * m : 128 * m + 128],
                            stg_list[kc][:, 512 * n : 512 * n + 512],
                            start=(kc == 0),
                            stop=(kc == 7),
                        )
                for n in range(4):
                    if is_q:
                        for sub in range(2):
                            pb = 64 * sub
                            nc.vector.tensor_scalar_add(
                                qTz[2 * m + sub][
                                    pb : pb + 64, 512 * n : 512 * n + 512
                                ],
                                psums[n][pb : pb + 64, :],
                                b_sb[pb : pb + 64, m : m + 1],
                            )
                    else:
                        nc.vector.tensor_scalar_add(
                            kTt[m][:, 512 * n : 512 * n + 512],
                            psums[n][:],
                            b_sb[:, m : m + 1],
                        )

            with tc.tile_pool(name="ppsum", bufs=4, space="PSUM") as pps:
                proj_phase(wq, stq, bq, True, 0, pps)
                proj_phase(wk, stk, bk, False, 0, pps)
                # heads 0/1 scores+exp start while m-tile 1 projects
                es_h0, lh_h0 = section_scores(0, 0, defer=True)
                es_h1, lh_h1 = section_scores(0, 1, defer=True)
                proj_phase(wq, stq, bq, True, 1, pps)
                proj_phase(wk, stk, bk, False, 1, pps)

            with tc.tile_pool(name="ypsum", bufs=1, space="PSUM") as psy:
                psy_pool[0] = psy
                # ---------------- v projection ----------------
                with tc.tile_pool(name="vpsum", bufs=2, space="PSUM") as psv:
                    for grp in range(2):
                        for i in range(8):
                            li = grp * 8 + i
                            pv = psv.tile([128, MPC], F32, tag="v")
                            for kc in range(8):
                                nc.tensor.matmul(
                                    pv[:],
                                    stv[kc][:, 128 * li : 128 * li + 128],
                                    wv[:, kc, :],
                                    start=(kc == 0),
                                    stop=(kc == 7),
                                )
                            nc.vector.tensor_copy(
                                vxg[li // 4][:, li % 4, :, 0:HD],
                                pv[:].rearrange("p (h d) -> p h d", h=HPC),
                            )

                # deferred AVs for heads 0/1, then the rest of qs0
                section_av(0, 0, es_h0, lh_h0)
                section_av(0, 1, es_h1, lh_h1)
                section_scores(0, 2, defer=False)
                section_scores(0, 3, defer=False)
                # first qs1 section hides the qs0 out-projection's deps
                section_scores(1, 0, defer=False)
                with tc.tile_pool(name="opsum", bufs=2, space="PSUM") as pso:
                    pso_pool[0] = pso
                    outproj(0)
                    section_scores(1, 1, defer=False)
                    section_scores(1, 2, defer=False)
                    section_scores(1, 3, defer=False)
                    _emit_norm(pending_norm[0])
                    pending_norm[0] = None
                    outproj(1)

    nc.finalize()
    return nc


def _host_tables(rel_emb: np.ndarray):
    """Per-core-group band tables; rel_emb is [NUM_BUCKETS, H]."""
    d = np.arange(4095)
    rp = d - 2047  # key - query
    buckets = _bucket(rp)
    bands = []
    c31s = []
    for h in range(H):
        vals = rel_emb[buckets, h].astype(np.float32)
        vals = np.where(rp > 0, np.float32(NEG), vals)  # causal mask
        band_pad = np.full(4223, NEG, np.float32)
        band_pad[:4095] = vals
        # BS[r, x] = band_pad[4095 + r - (x + 1152)]  (cols < 1152 unused)
        idx = 4095 + np.arange(128)[:, None] - np.arange(1152, 4096)[None, :]
        bands.append(band_pad[idx].astype(BF_NP))
        c31s.append(np.float32(rel_emb[31, h]))
    return bands, c31s


def _numpy_ref(query, key, value, attn_mask, key_padding_mask,
               Wq, bq, Wk, bk, Wv, bv, Wo, bo, rel_emb):
    """Exact numpy fallback for unexpected mask patterns."""
    q = (query @ Wq.T + bq).reshape(B, L, H, HD).transpose(0, 2, 1, 3)
    k = (key @ Wk.T + bk).reshape(B, L, H, HD).transpose(0, 2, 1, 3)
    v = (value @ Wv.T + bv).reshape(B, L, H, HD).transpose(0, 2, 1, 3)
    scores = np.einsum("bhqd,bhkd->bhqk", q, k) / math.sqrt(HD)
    rp = np.arange(L, dtype=np.int64)[None, :] - np.arange(L, dtype=np.int64)[:, None]
    rel = rel_emb[_bucket(rp)].transpose(2, 0, 1)
    scores = scores + rel[None]
    scores = np.where(attn_mask[None, None], scores, -np.inf)
    scores = np.where(key_padding_mask[:, None, None, :], scores, -np.inf)
    scores = scores - scores.max(-1, keepdims=True)
    e = np.exp(scores)
    attn = e / e.sum(-1, keepdims=True)
    out = np.einsum("bhqk,bhkd->bhqd", attn, v)
    out = out.transpose(0, 2, 1, 3).reshape(B, L, D)
    return (out @ Wo.T + bo).astype(np.float32)


def kernel(**inputs) -> np.ndarray:
    global _cached, last_results
    inp = {k: np.asarray(v) for k, v in inputs.items()}
    query, key, value = inp["query"], inp["key"], inp["value"]
    attn_mask, kpm = inp["attn_mask"], inp["key_padding_mask"]
    Wq, bq, Wk, bk = inp["Wq"], inp["bq"], inp["Wk"], inp["bk"]
    Wv, bv, Wo, bo = inp["Wv"], inp["bv"], inp["Wo"], inp["bo"]
    rel_emb = inp["rel_emb"]

    causal = np.array_equal(attn_mask, np.tril(np.ones((L, L), bool)))
    if not (causal and kpm.all()):
        return _numpy_ref(**inp)

    if _cached is None:
        _cached = _build()
    nc = _cached

    bands, c31s = _host_tables(rel_emb)
    ident = np.eye(128, dtype=BF_NP)

    def _rearr_w(w_slice):  # [MPC, D] row-major weights -> [128, 8, MPC]
        arr = np.ascontiguousarray(w_slice.T)  # [D, MPC]
        return arr.reshape(8, 128, MPC).transpose(1, 0, 2).astype(BF_NP)

    in_maps = []
    for c in range(N_CORES):
        b, hg = c // HPC, c % HPC
        rows = slice(MPC * hg, MPC * hg + MPC)
        heads = range(HPC * hg, HPC * hg + HPC)
        wo_c = np.ascontiguousarray(Wo[:, rows].T)  # [MPC, D]
        in_maps.append({
            "qT_in": np.ascontiguousarray(query[b].T).astype(BF_NP),
            "kT_in": np.ascontiguousarray(key[b].T).astype(BF_NP),
            "vT_in": np.ascontiguousarray(value[b].T).astype(BF_NP),
            "wq_in": _rearr_w(Wq[rows] / math.sqrt(HD)),
            "wk_in": _rearr_w(Wk[rows]),
            "wv_in": _rearr_w(Wv[rows]),
            "wo_in": wo_c.reshape(2, 128, D).transpose(1, 0, 2).astype(BF_NP),
            "bq_in": np.ascontiguousarray(
                (bq[rows] / math.sqrt(HD)).reshape(2, 128).T.astype(np.float32)
            ),
            "bk_in": np.ascontiguousarray(
                bk[rows].reshape(2, 128).T.astype(np.float32)
            ),
            "band_in": np.stack([bands[h] for h in heads]),
            "c31_in": np.tile(
                np.array([c31s[h] for h in heads], np.float32), (128, 1)
            ),
            "id_in": ident,
        })

    res = run_bass_kernel_spmd(nc, in_maps, list(range(N_CORES)))
    last_results = res

    bo_eff = (
        bo.astype(np.float64) + bv.astype(np.float64) @ Wo.T.astype(np.float64)
    )
    out = np.empty((B, L, D), np.float32)
    for b in range(B):
        acc = np.zeros((D, L), np.float64)
        for hg in range(HPC):
            part = res.results[b * HPC + hg]["outT"].astype(np.float64)
            acc += part.transpose(1, 0, 2).reshape(D, L)
        out[b] = (acc.T + bo_eff[None, :]).astype(np.float32)
    return out


# revision 25
# speedup vs baseline: 1.3334x; 1.0611x over previous
"""Bass/Trainium2 kernel for nn_MultiHeadAttention (T5-style rel-bias causal MHA).

Sharding: 8 cores = 2 batches x 4 head-groups (4 heads of 64 dims each).
Each core: projects q/k/v for its 256 proj rows (bf16 operands, fp32 PSUM),
runs causal attention with the T5 relative bias folded in as either a PSUM
band preload (near-diagonal blocks) or a constant exp-bias
(bucket-31-saturated blocks), and computes a partial out-projection.
Host sums the 4 bf16 partials per batch.

v2 layout: bf16 operands everywhere (halves HBM traffic vs f32r), m-tile
ordered q/k projection so attention sections for heads 0/1 start early
(their AV deferred until v-proj lands), qs-outer section order with the
qs0 out-projection overlapped under qs1 attention, and the scalar engine
kept free of DMA issue so it does nothing but exp.
"""
import math
import sys

sys.path.insert(0, "/opt/trn_rl_repo")

import numpy as np
import ml_dtypes

from concourse import bacc
import concourse.mybir as mybir
import concourse.tile as tile
from concourse.bass_utils import run_bass_kernel_spmd

F32 = mybir.dt.float32
F32R = mybir.dt.float32r
BF = mybir.dt.bfloat16
Exp = mybir.ActivationFunctionType.Exp
Copy = mybir.ActivationFunctionType.Copy
MUL = mybir.AluOpType.mult

B, L, D = 2, 2048, 1024
H, HD = 16, 64
NUM_BUCKETS, MAX_DISTANCE = 32, 128
HPC = 4  # heads per core
MPC = HPC * HD  # 256 proj rows per core
N_CORES = 8
NEG = -60.0  # additive mask value (exp(-60+s) == 0 in practice)
BF_NP = ml_dtypes.bfloat16

last_results = None  # BassKernelResults of the most recent run (for profiling)
_cached = None


def _bucket(rp: np.ndarray) -> np.ndarray:
    """T5 relative position bucket, mirrors the reference exactly."""
    sign = (rp > 0).astype(np.int32)
    n = np.abs(rp)
    max_exact = NUM_BUCKETS // 2
    n_safe = np.maximum(n, 1).astype(np.float32)
    vil = max_exact + (
        np.log(n_safe / max_exact)
        / math.log(MAX_DISTANCE / max_exact)
        * (NUM_BUCKETS - max_exact)
    ).astype(np.int32)
    vil = np.minimum(vil, NUM_BUCKETS - 1)
    buckets = np.where(n < max_exact, n, vil) + sign * max_exact
    return np.clip(buckets, 0, NUM_BUCKETS - 1)


def _build():
    nc = bacc.Bacc(trn_type="TRN2")

    qT_in = nc.dram_tensor("qT_in", [D, L], BF, kind="ExternalInput")
    kT_in = nc.dram_tensor("kT_in", [D, L], BF, kind="ExternalInput")
    vT_in = nc.dram_tensor("vT_in", [D, L], BF, kind="ExternalInput")
    wq_in = nc.dram_tensor("wq_in", [128, 8, MPC], BF, kind="ExternalInput")
    wk_in = nc.dram_tensor("wk_in", [128, 8, MPC], BF, kind="ExternalInput")
    wv_in = nc.dram_tensor("wv_in", [128, 8, MPC], BF, kind="ExternalInput")
    wo_in = nc.dram_tensor("wo_in", [128, 2, D], BF, kind="ExternalInput")
    bq_in = nc.dram_tensor("bq_in", [128, 2], F32, kind="ExternalInput")
    bk_in = nc.dram_tensor("bk_in", [128, 2], F32, kind="ExternalInput")
    # band columns 0..1151 are never addressed (x0 >= 1152): trimmed
    band_in = nc.dram_tensor("band_in", [HPC, 128, 2944], BF, kind="ExternalInput")
    c31_in = nc.dram_tensor("c31_in", [128, HPC], F32, kind="ExternalInput")
    id_in = nc.dram_tensor("id_in", [128, 128], BF, kind="ExternalInput")
    # [p, n, l] layout: row (128n + p) of the [D, L] partial lives at [p, n, l]
    outT = nc.dram_tensor("outT", [128, 8, L], BF, kind="ExternalOutput")

    with tile.TileContext(nc) as tc:
        with (
            tc.tile_pool(name="res", bufs=1) as pr,
            tc.tile_pool(name="qkv", bufs=1) as pqkv,
            tc.tile_pool(name="stg", bufs=16) as pstg,
            tc.tile_pool(name="es", bufs=16) as pes,
            tc.tile_pool(name="misc", bufs=2) as pmisc,
            tc.tile_pool(name="spsum", bufs=2, space="PSUM") as psc,
        ):
            # ---- input DMAs: qT chunks on sync, everything else gpsimd ----
            # scalar's HWDGE ring is idle until the first exp (~30us), so
            # odd chunks ride it for 2x input stream rate during proj.
            stq = []
            for kc in range(8):
                t = pstg.tile([128, L], BF, tag="stage", name=f"sq{kc}")
                eng = nc.sync if kc % 2 == 0 else nc.scalar
                eng.dma_start(t[:], qT_in[128 * kc : 128 * kc + 128, :])
                stq.append(t)
            stk = []
            for kc in range(8):
                t = pstg.tile([128, L], BF, tag="stage", name=f"sk{kc}")
                eng = nc.sync if kc % 2 == 0 else nc.scalar
                eng.dma_start(t[:], kT_in[128 * kc : 128 * kc + 128, :])
                stk.append(t)
            wq = pr.tile([128, 8, MPC], BF)
            nc.gpsimd.dma_start(wq[:], wq_in[:])
            wk = pr.tile([128, 8, MPC], BF)
            nc.gpsimd.dma_start(wk[:], wk_in[:])
            bq = pr.tile([128, 2], F32)
            nc.gpsimd.dma_start(bq[:], bq_in[:])
            bk = pr.tile([128, 2], F32)
            nc.gpsimd.dma_start(bk[:], bk_in[:])
            c31 = pr.tile([128, HPC], F32)
            nc.gpsimd.dma_start(c31[:], c31_in[:])
            ident = pr.tile([128, 128], BF)
            nc.gpsimd.dma_start(ident[:], id_in[:])
            bands = []
            for h in range(HPC):
                t = pr.tile([128, 2944], BF, name=f"band{h}")
                nc.gpsimd.dma_start(t[:], band_in[h])
                bands.append(t)
            # v input staged behind q (reuses q's stage bufs), issued on sync
            stv = []
            for kc in range(8):
                t = pstg.tile([128, L], BF, tag="stage", name=f"sv{kc}")
                nc.sync.dma_start(t[:], vT_in[128 * kc : 128 * kc + 128, :])
                stv.append(t)
            wv = pr.tile([128, 8, MPC], BF)
            nc.gpsimd.dma_start(wv[:], wv_in[:])
            wo = pr.tile([128, 2, D], BF)
            nc.gpsimd.dma_start(wo[:], wo_in[:])

            # warm the ACT exp table early, off the critical path
            warm = pr.tile([1, 2], F32)
            nc.vector.memset(warm[:], 0.0)
            nc.scalar.activation(warm[:], warm[:], Exp)
            ones_v = pr.tile([1, HD], BF)
            nc.vector.memset(ones_v[:], 1.0)

            qTz = []
            for hh in range(HPC):
                t = pqkv.tile([128, L], BF, name=f"qtz{hh}")
                nc.vector.memset(t[:].bitcast(F32), 0)
                qTz.append(t)
            kTt = [pqkv.tile([128, L], BF, name=f"kt{mm}") for mm in range(2)]
            vxg = []
            for g in range(4):
                t = pqkv.tile([128, 4, HPC, HD + 1], BF, name=f"vx{g}")
                nc.vector.memset(t[:, :, :, HD], 1.0)
                vxg.append(t)
            y_norm_qs = [
                pqkv.tile([128, 2, 1024], BF, name=f"yn{qq}") for qq in range(2)
            ]

            # ---------------- attention section machinery ----------------
            pending_norm = [None]

            def _emit_norm(item):
                # PE-side replication of the reciprocal row + in-place
                # multiply; prep rides the score-psum ring.
                rrow, pb, mt, qsi = item
                prep = psc.tile([128, 1024], F32, tag="score", name="prep")
                nc.tensor.matmul(
                    prep[0:HD, :512], ones_v[:], rrow[:, :512],
                    start=True, stop=True,
                )
                nc.tensor.matmul(
                    prep[0:HD, 512:], ones_v[:], rrow[:, 512:],
                    start=True, stop=True,
                )
                prep_sb = pmisc.tile([128, 1024], BF, tag="prep")
                nc.vector.tensor_copy(prep_sb[pb : pb + 64, :], prep[0:HD, :])
                nc.vector.tensor_tensor(
                    y_norm_qs[qsi][pb : pb + 64, mt, :],
                    y_norm_qs[qsi][pb : pb + 64, mt, :],
                    prep_sb[pb : pb + 64, :],
                    MUL,
                )

            def section_scores(qs, h, defer):
                """Emit preload+score MMs and exp for section (qs, h).
                If defer, skip AV (return the es list); else pipeline AV."""
                mt = h // 2
                q0 = 1024 * qs
                n_live = 8 * (qs + 1)
                live_half = [min(4 * (2 * qs + j + 1), 16) for j in (0, 1)]
                es_list = []
                yT = None
                if not defer:
                    yT = psy_pool[0].tile([HD + 1, 1024], F32, tag="yT")
                pending = None
                for ki in range(n_live):
                    const_blk = 128 * ki <= q0 - 240
                    halves = [j for j in (0, 1) if ki < live_half[j]]
                    # causal staircase: query-columns below the diagonal are
                    # dead; narrow the moving operand to the live range.
                    c0s = {
                        j: max(0, 128 * ki - (q0 + 512 * j)) for j in halves
                    }
                    sp = psc.tile([128, 1024], F32, tag="score")
                    for j in halves:
                        a = 512 * j + c0s[j]
                        b = 512 * j + 512
                        if not const_blk:
                            x0 = 2048 - 128 * ki + q0 - 1152 + a
                            nc.tensor.matmul(
                                sp[:, a:b],
                                ident[:],
                                bands[h][:, x0 : x0 + (b - a)],
                                start=True,
                                stop=False,
                            )
                        nc.tensor.matmul(
                            sp[:, a:b],
                            kTt[mt][:, 128 * ki : 128 * ki + 128],
                            qTz[h][:, q0 + a : q0 + b],
                            start=const_blk,
                            stop=True,
                        )
                    es = pes.tile([128, 1024], BF, tag="es")
                    bias = c31[:, h : h + 1] if const_blk else 0.0
                    # live ranges of the two halves are always contiguous
                    lo = min(512 * j + c0s[j] for j in halves)
                    hi = 512 * halves[-1] + 512
                    nc.scalar.activation(
                        es[:, lo:hi], sp[:, lo:hi], Exp, bias=bias
                    )
                    es_list.append((es, halves, ki))
                    if not defer:
                        if pending is not None:
                            _av_one(yT, h, pending, live_half, qs)
                        pending = (es, halves, ki)
                if not defer:
                    _av_one(yT, h, pending, live_half, qs)
                    _finish_section(yT, qs, h)
                return es_list, live_half

            def _av_one(yT, h, item, live_half, qs):
                es, halves, ki = item
                q0 = 1024 * qs
                for j in halves:
                    a = 512 * j + max(0, 128 * ki - (q0 + 512 * j))
                    b = 512 * j + 512
                    nc.tensor.matmul(
                        yT[:, a:b],
                        vxg[ki // 4][:, ki % 4, h, :],
                        es[:, a:b],
                        start=(ki == 0),
                        stop=(ki == live_half[j] - 1),
                    )

            def section_av(qs, h, es_list, live_half):
                yT = psy_pool[0].tile([HD + 1, 1024], F32, tag="yT")
                for item in es_list:
                    _av_one(yT, h, item, live_half, qs)
                _finish_section(yT, qs, h)

            def _finish_section(yT, qs, h):
                # evacuate yT (unnormalized) into its y_norm slot; the
                # denominator row goes out via ACT so DVE+ACT overlap and
                # the psy buffer frees fast.  The reciprocal chain's
                # replication+multiply for the PREVIOUS section is emitted
                # now (its rrow is long ready), so the PE never stalls on
                # the recip chain.
                mt = h // 2
                pb = 64 * (h % 2)
                dcp = pmisc.tile([1, 1024], F32, tag="dcp")
                nc.vector.tensor_copy(dcp[:], yT[HD : HD + 1, :])
                nc.vector.tensor_copy(
                    y_norm_qs[qs][pb : pb + 64, mt, :], yT[0:HD, :]
                )
                dT = pmisc.tile([128, 8], F32, tag="dT")
                nc.sync.dma_start(dT[:], dcp[:])
                rT = pmisc.tile([128, 8], BF, tag="rT")
                with nc.allow_low_precision(reason="softmax recip bf16"):
                    nc.vector.reciprocal(rT[:], dT[:])
                rrow = pmisc.tile([1, 1024], BF, tag="rrow")
                nc.sync.dma_start(rrow[:], rT[:])
                if pending_norm[0] is not None:
                    _emit_norm(pending_norm[0])
                pending_norm[0] = (rrow, pb, mt, qs)

            def outproj(qs):
                # stage the whole 512-query slab in SBUF, write it with ONE
                # DMA: avoids the ~2us per-DMA completion cost serializing
                # the PSUM ring at the tail.
                for sl in range(2):
                    qi = 2 * qs + sl
                    ost = pmisc.tile([128, 8, 512], BF, tag="ost")
                    for n in range(8):
                        po = pso_pool[0].tile([128, 512], F32, tag="out")
                        for c in range(2):
                            nc.tensor.matmul(
                                po[:],
                                wo[:, c, 128 * n : 128 * n + 128],
                                y_norm_qs[qs][:, c, 512 * sl : 512 * sl + 512],
                                start=(c == 0),
                                stop=(c == 1),
                            )
                        nc.vector.tensor_copy(ost[:, n, :], po[:])
                    nc.sync.dma_start(
                        outT[:, :, 512 * qi : 512 * qi + 512], ost[:]
                    )

            # ---------------- q/k projections (m-tile major) ----------------
            psy_pool = [None]
            pso_pool = [None]

            def proj_phase(w_sb, stg_list, b_sb, is_q, m, pool):
                psums = [
                    pool.tile([128, 512], F32, tag="qk", name=f"p{m}{n}")
                    for n in range(4)
                ]
                for kc in range(8):
                    for n in range(4):
                        nc.tensor.matmul(
                            psums[n][:],
                            w_sb[:, kc, 128 * m : 128 * m + 128],
                            stg_list[kc][:, 512 * n : 512 * n + 512],
                            start=(kc == 0),
                            stop=(kc == 7),
                        )
                for n in range(4):
                    if is_q:
                        for sub in range(2):
                            pb = 64 * sub
                            nc.vector.tensor_scalar_add(
                                qTz[2 * m + sub][
                                    pb : pb + 64, 512 * n : 512 * n + 512
                                ],
                                psums[n][pb : pb + 64, :],
                                b_sb[pb : pb + 64, m : m + 1],
                            )
                    else:
                        nc.vector.tensor_scalar_add(
                            kTt[m][:, 512 * n : 512 * n + 512],
                            psums[n][:],
                            b_sb[:, m : m + 1],
                        )

            with tc.tile_pool(name="ppsum", bufs=4, space="PSUM") as pps:
                proj_phase(wq, stq, bq, True, 0, pps)
                proj_phase(wk, stk, bk, False, 0, pps)
                # heads 0/1 scores+exp start while m-tile 1 projects
                es_h0, lh_h0 = section_scores(0, 0, defer=True)
                es_h1, lh_h1 = section_scores(0, 1, defer=True)
                proj_phase(wq, stq, bq, True, 1, pps)
                proj_phase(wk, stk, bk, False, 1, pps)

            with tc.tile_pool(name="ypsum", bufs=1, space="PSUM") as psy:
                psy_pool[0] = psy
                # ---------------- v projection ----------------
                with tc.tile_pool(name="vpsum", bufs=2, space="PSUM") as psv:
                    for grp in range(2):
                        for i in range(8):
                            li = grp * 8 + i
                            pv = psv.tile([128, MPC], F32, tag="v")
                            for kc in range(8):
                                nc.tensor.matmul(
                                    pv[:],
                                    stv[kc][:, 128 * li : 128 * li + 128],
                                    wv[:, kc, :],
                                    start=(kc == 0),
                                    stop=(kc == 7),
                                )
                            nc.vector.tensor_copy(
                                vxg[li // 4][:, li % 4, :, 0:HD],
                                pv[:].rearrange("p (h d) -> p h d", h=HPC),
                            )

                # deferred AVs for heads 0/1, then the rest of qs0
                section_av(0, 0, es_h0, lh_h0)
                section_av(0, 1, es_h1, lh_h1)
                section_scores(0, 2, defer=False)
                section_scores(0, 3, defer=False)
                # first qs1 section hides the qs0 out-projection's deps
                section_scores(1, 0, defer=False)
                with tc.tile_pool(name="opsum", bufs=2, space="PSUM") as pso:
                    pso_pool[0] = pso
                    outproj(0)
                    section_scores(1, 1, defer=False)
                    section_scores(1, 2, defer=False)
                    section_scores(1, 3, defer=False)
                    _emit_norm(pending_norm[0])
                    pending_norm[0] = None
                    outproj(1)

    nc.finalize()
    return nc


def _host_tables(rel_emb: np.ndarray):
    """Per-core-group band tables; rel_emb is [NUM_BUCKETS, H]."""
    d = np.arange(4095)
    rp = d - 2047  # key - query
    buckets = _bucket(rp)
    bands = []
    c31s = []
    for h in range(H):
        vals = rel_emb[buckets, h].astype(np.float32)
        vals = np.where(rp > 0, np.float32(NEG), vals)  # causal mask
        band_pad = np.full(4223, NEG, np.float32)
        band_pad[:4095] = vals
        # BS[r, x] = band_pad[4095 + r - (x + 1152)]  (cols < 1152 unused)
        idx = 4095 + np.arange(128)[:, None] - np.arange(1152, 4096)[None, :]
        bands.append(band_pad[idx].astype(BF_NP))
        c31s.append(np.float32(rel_emb[31, h]))
    return bands, c31s


def _numpy_ref(query, key, value, attn_mask, key_padding_mask,
               Wq, bq, Wk, bk, Wv, bv, Wo, bo, rel_emb):
    """Exact numpy fallback for unexpected mask patterns."""
    q = (query @ Wq.T + bq).reshape(B, L, H, HD).transpose(0, 2, 1, 3)
    k = (key @ Wk.T + bk).reshape(B, L, H, HD).transpose(0, 2, 1, 3)
    v = (value @ Wv.T + bv).reshape(B, L, H, HD).transpose(0, 2, 1, 3)
    scores = np.einsum("bhqd,bhkd->bhqk", q, k) / math.sqrt(HD)
    rp = np.arange(L, dtype=np.int64)[None, :] - np.arange(L, dtype=np.int64)[:, None]
    rel = rel_emb[_bucket(rp)].transpose(2, 0, 1)
    scores = scores + rel[None]
    scores = np.where(attn_mask[None, None], scores, -np.inf)
    scores = np.where(key_padding_mask[:, None, None, :], scores, -np.inf)
    scores = scores - scores.max(-1, keepdims=True)
    e = np.exp(scores)
    attn = e / e.sum(-1, keepdims=True)
    out = np.einsum("bhqk,bhkd->bhqd", attn, v)
    out = out.transpose(0, 2, 1, 3).reshape(B, L, D)
    return (out @ Wo.T + bo).astype(np.float32)


def kernel(**inputs) -> np.ndarray:
    global _cached, last_results
    inp = {k: np.asarray(v) for k, v in inputs.items()}
    query, key, value = inp["query"], inp["key"], inp["value"]
    attn_mask, kpm = inp["attn_mask"], inp["key_padding_mask"]
    Wq, bq, Wk, bk = inp["Wq"], inp["bq"], inp["Wk"], inp["bk"]
    Wv, bv, Wo, bo = inp["Wv"], inp["bv"], inp["Wo"], inp["bo"]
    rel_emb = inp["rel_emb"]

    causal = np.array_equal(attn_mask, np.tril(np.ones((L, L), bool)))
    if not (causal and kpm.all()):
        return _numpy_ref(**inp)

    if _cached is None:
        _cached = _build()
    nc = _cached

    bands, c31s = _host_tables(rel_emb)
    ident = np.eye(128, dtype=BF_NP)

    def _rearr_w(w_slice):  # [MPC, D] row-major weights -> [128, 8, MPC]
        arr = np.ascontiguousarray(w_slice.T)  # [D, MPC]
        return arr.reshape(8, 128, MPC).transpose(1, 0, 2).astype(BF_NP)

    in_maps = []
    for c in range(N_CORES):
        b, hg = c // HPC, c % HPC
        rows = slice(MPC * hg, MPC * hg + MPC)
        heads = range(HPC * hg, HPC * hg + HPC)
        wo_c = np.ascontiguousarray(Wo[:, rows].T)  # [MPC, D]
        in_maps.append({
            "qT_in": np.ascontiguousarray(query[b].T).astype(BF_NP),
            "kT_in": np.ascontiguousarray(key[b].T).astype(BF_NP),
            "vT_in": np.ascontiguousarray(value[b].T).astype(BF_NP),
            "wq_in": _rearr_w(Wq[rows] / math.sqrt(HD)),
            "wk_in": _rearr_w(Wk[rows]),
            "wv_in": _rearr_w(Wv[rows]),
            "wo_in": wo_c.reshape(2, 128, D).transpose(1, 0, 2).astype(BF_NP),
            "bq_in": np.ascontiguousarray(
                (bq[rows] / math.sqrt(HD)).reshape(2, 128).T.astype(np.float32)
            ),
            "bk_in": np.ascontiguousarray(
                bk[rows].reshape(2, 128).T.astype(np.float32)
            ),
            "band_in": np.stack([bands[h] for h in heads]),
            "c31_in": np.tile(
                np.array([c31s[h] for h in heads], np.float32), (128, 1)
            ),
            "id_in": ident,
        })

    res = run_bass_kernel_spmd(nc, in_maps, list(range(N_CORES)))
    last_results = res

    bo_eff = (
        bo.astype(np.float64) + bv.astype(np.float64) @ Wo.T.astype(np.float64)
    )
    out = np.empty((B, L, D), np.float32)
    for b in range(B):
        acc = np.zeros((D, L), np.float64)
        for hg in range(HPC):
            part = res.results[b * HPC + hg]["outT"].astype(np.float64)
            acc += part.transpose(1, 0, 2).reshape(D, L)
        out[b] = (acc.T + bo_eff[None, :]).astype(np.float32)
    return out


# revision 34
# speedup vs baseline: 1.5333x; 1.1500x over previous
"""Bass/Trainium2 kernel for nn_MultiHeadAttention (T5-style rel-bias causal MHA).

Sharding: 8 cores = 2 batches x 4 head-groups (4 heads of 64 dims each).
Each core: projects q/k/v for its 256 proj rows (bf16 operands, fp32 PSUM),
runs causal attention with the T5 relative bias folded in as either a PSUM
band preload (near-diagonal blocks) or a constant exp-bias
(bucket-31-saturated blocks), and computes a partial out-projection.
Host sums the 4 bf16 partials per batch.

v2 layout: bf16 operands everywhere (halves HBM traffic vs f32r), m-tile
ordered q/k projection so attention sections for heads 0/1 start early
(their AV deferred until v-proj lands), qs-outer section order with the
qs0 out-projection overlapped under qs1 attention, and the scalar engine
kept free of DMA issue so it does nothing but exp.
"""
import math
import sys

sys.path.insert(0, "/opt/trn_rl_repo")

import numpy as np
import ml_dtypes

from concourse import bacc
import concourse.mybir as mybir
import concourse.tile as tile
from concourse.bass_utils import run_bass_kernel_spmd

F32 = mybir.dt.float32
F32R = mybir.dt.float32r
BF = mybir.dt.bfloat16
Exp = mybir.ActivationFunctionType.Exp
Copy = mybir.ActivationFunctionType.Copy
MUL = mybir.AluOpType.mult

B, L, D = 2, 2048, 1024
H, HD = 16, 64
NUM_BUCKETS, MAX_DISTANCE = 32, 128
HPC = 4  # heads per core
MPC = HPC * HD  # 256 proj rows per core
N_CORES = 8
NEG = -60.0  # additive mask value (exp(-60+s) == 0 in practice)
BF_NP = ml_dtypes.bfloat16
# per-qs: number of live 128-key blocks for each 512-query half
LIVE_HALF = {
    qs: [min(4 * (2 * qs + j + 1), 16) for j in (0, 1)] for qs in (0, 1)
}

last_results = None  # BassKernelResults of the most recent run (for profiling)
_cached = None


def _bucket(rp: np.ndarray) -> np.ndarray:
    """T5 relative position bucket, mirrors the reference exactly."""
    sign = (rp > 0).astype(np.int32)
    n = np.abs(rp)
    max_exact = NUM_BUCKETS // 2
    n_safe = np.maximum(n, 1).astype(np.float32)
    vil = max_exact + (
        np.log(n_safe / max_exact)
        / math.log(MAX_DISTANCE / max_exact)
        * (NUM_BUCKETS - max_exact)
    ).astype(np.int32)
    vil = np.minimum(vil, NUM_BUCKETS - 1)
    buckets = np.where(n < max_exact, n, vil) + sign * max_exact
    return np.clip(buckets, 0, NUM_BUCKETS - 1)


def _build():
    nc = bacc.Bacc(trn_type="TRN2")

    qT_in = nc.dram_tensor("qT_in", [D, L], BF, kind="ExternalInput")
    kT_in = nc.dram_tensor("kT_in", [D, L], BF, kind="ExternalInput")
    vT_in = nc.dram_tensor("vT_in", [D, L], BF, kind="ExternalInput")
    wq_in = nc.dram_tensor("wq_in", [128, 8, MPC], BF, kind="ExternalInput")
    wk_in = nc.dram_tensor("wk_in", [128, 8, MPC], BF, kind="ExternalInput")
    wv_in = nc.dram_tensor("wv_in", [128, 8, MPC], BF, kind="ExternalInput")
    wo_in = nc.dram_tensor("wo_in", [128, 2, D], BF, kind="ExternalInput")
    bq_in = nc.dram_tensor("bq_in", [128, 2], F32, kind="ExternalInput")
    bk_in = nc.dram_tensor("bk_in", [128, 2], F32, kind="ExternalInput")
    # band columns 0..1151 are never addressed (x0 >= 1152): trimmed
    band_in = nc.dram_tensor("band_in", [HPC, 128, 2944], BF, kind="ExternalInput")
    c31_in = nc.dram_tensor("c31_in", [128, HPC], F32, kind="ExternalInput")
    id_in = nc.dram_tensor("id_in", [128, 128], BF, kind="ExternalInput")
    # [p, n, l] layout: row (128n + p) of the [D, L] partial lives at [p, n, l]
    outT = nc.dram_tensor("outT", [128, 8, L], BF, kind="ExternalOutput")

    with tile.TileContext(nc) as tc:
        with (
            tc.tile_pool(name="res", bufs=1) as pr,
            tc.tile_pool(name="qkv", bufs=1) as pqkv,
            tc.tile_pool(name="stg", bufs=16) as pstg,
            tc.tile_pool(name="es", bufs=16) as pes,
            tc.tile_pool(name="misc", bufs=2) as pmisc,
        ):
            # ---- input DMAs ----
            # One serialized full-bandwidth stream on sync in consumption
            # order (qT, kT, vT) so the PE is never starved by fair-shared
            # HBM; weights/consts/bands ride gpsimd (SWDGE); scalar issues
            # nothing (kept pure-exp).
            stq = []
            for kc in range(8):
                t = pstg.tile([128, L], BF, tag="stage", name=f"sq{kc}")
                nc.sync.dma_start(t[:], qT_in[128 * kc : 128 * kc + 128, :])
                stq.append(t)
            stk = []
            for kc in range(8):
                t = pstg.tile([128, L], BF, tag="stage", name=f"sk{kc}")
                nc.sync.dma_start(t[:], kT_in[128 * kc : 128 * kc + 128, :])
                stk.append(t)
            stv = []
            for kc in range(8):
                t = pstg.tile([128, L], BF, tag="stage", name=f"sv{kc}")
                nc.sync.dma_start(t[:], vT_in[128 * kc : 128 * kc + 128, :])
                stv.append(t)
            # gpsimd: wq first (the first proj MM gates on it)
            wq = pr.tile([128, 8, MPC], BF)
            nc.gpsimd.dma_start(wq[:], wq_in[:])
            bq = pr.tile([128, 2], F32)
            nc.gpsimd.dma_start(bq[:], bq_in[:])
            wk = pr.tile([128, 8, MPC], BF)
            nc.gpsimd.dma_start(wk[:], wk_in[:])
            bk = pr.tile([128, 2], F32)
            nc.gpsimd.dma_start(bk[:], bk_in[:])
            wv = pr.tile([128, 8, MPC], BF)
            nc.gpsimd.dma_start(wv[:], wv_in[:])
            c31 = pr.tile([128, HPC], F32)
            nc.gpsimd.dma_start(c31[:], c31_in[:])
            ident = pr.tile([128, 128], BF)
            nc.gpsimd.dma_start(ident[:], id_in[:])
            wo = pr.tile([128, 2, D], BF)
            nc.gpsimd.dma_start(wo[:], wo_in[:])
            bands = []
            for h in range(HPC):
                t = pr.tile([128, 2944], BF, name=f"band{h}")
                nc.gpsimd.dma_start(t[:], band_in[h])
                bands.append(t)

            # warm the ACT exp table early, off the critical path
            warm = pr.tile([1, 2], F32)
            nc.vector.memset(warm[:], 0.0)
            nc.scalar.activation(warm[:], warm[:], Exp)
            ones_v = pr.tile([1, HD], BF)
            nc.vector.memset(ones_v[:], 1.0)

            qTz = []
            for hh in range(HPC):
                t = pqkv.tile([128, L], BF, name=f"qtz{hh}")
                nc.vector.memset(t[:].bitcast(F32), 0)
                qTz.append(t)
            kTt = [pqkv.tile([128, L], BF, name=f"kt{mm}") for mm in range(2)]
            vxg = []
            for g in range(4):
                t = pqkv.tile([128, 4, HPC, HD + 1], BF, name=f"vx{g}")
                nc.vector.memset(t[:, :, :, HD], 1.0)
                vxg.append(t)
            y_norm_qs = [
                pqkv.tile([128, 2, 1024], BF, name=f"yn{qq}") for qq in range(2)
            ]

            # ---------------- attention section machinery ----------------
            pending_norm = [None]

            def _emit_norm(item):
                # PE-side replication of the reciprocal row + in-place
                # multiply; prep rides the score-psum ring.
                rrow, pb, mt, qsi = item
                prep = psc.tile([128, 1024], F32, tag="score", name="prep")
                nc.tensor.matmul(
                    prep[0:HD, :512], ones_v[:], rrow[:, :512],
                    start=True, stop=True,
                )
                nc.tensor.matmul(
                    prep[0:HD, 512:], ones_v[:], rrow[:, 512:],
                    start=True, stop=True,
                )
                prep_sb = pmisc.tile([128, 1024], BF, tag="prep")
                nc.vector.tensor_copy(prep_sb[pb : pb + 64, :], prep[0:HD, :])
                nc.vector.tensor_tensor(
                    y_norm_qs[qsi][pb : pb + 64, mt, :],
                    y_norm_qs[qsi][pb : pb + 64, mt, :],
                    prep_sb[pb : pb + 64, :],
                    MUL,
                )

            def scores_gen(qs, h, es_list, ki_lo, ki_hi):
                """Generator: emit preload+score MMs and exp for section
                (qs, h), one ki per yield; AV is emitted separately."""
                mt = h // 2
                q0 = 1024 * qs
                for ki in range(ki_lo, ki_hi):
                    const_blk = 128 * ki <= q0 - 240
                    live_half = LIVE_HALF[qs]
                    halves = [j for j in (0, 1) if ki < live_half[j]]
                    # causal staircase: query-columns below the diagonal are
                    # dead; narrow the moving operand to the live range.
                    c0s = {
                        j: max(0, 128 * ki - (q0 + 512 * j)) for j in halves
                    }
                    sp = psc.tile([128, 1024], F32, tag="score")
                    for j in halves:
                        a = 512 * j + c0s[j]
                        b = 512 * j + 512
                        if not const_blk:
                            x0 = 2048 - 128 * ki + q0 - 1152 + a
                            nc.tensor.matmul(
                                sp[:, a:b],
                                ident[:],
                                bands[h][:, x0 : x0 + (b - a)],
                                start=True,
                                stop=False,
                            )
                        nc.tensor.matmul(
                            sp[:, a:b],
                            kTt[mt][:, 128 * ki : 128 * ki + 128],
                            qTz[h][:, q0 + a : q0 + b],
                            start=const_blk,
                            stop=True,
                        )
                    es = pes.tile([128, 1024], BF, tag="es")
                    bias = c31[:, h : h + 1] if const_blk else 0.0
                    # live ranges of the two halves are always contiguous
                    lo = min(512 * j + c0s[j] for j in halves)
                    hi = 512 * halves[-1] + 512
                    nc.scalar.activation(
                        es[:, lo:hi], sp[:, lo:hi], Exp, bias=bias
                    )
                    es_list.append((es, halves, ki))
                    yield

            def _av_one(yT, h, item, live_half, qs):
                es, halves, ki = item
                q0 = 1024 * qs
                for j in halves:
                    a = 512 * j + max(0, 128 * ki - (q0 + 512 * j))
                    b = 512 * j + 512
                    nc.tensor.matmul(
                        yT[:, a:b],
                        vxg[ki // 4][:, ki % 4, h, :],
                        es[:, a:b],
                        start=(ki == 0),
                        stop=(ki == live_half[j] - 1),
                    )

            def av_gen(qs, h, es_list):
                yT = psy_pool[0].tile([HD + 1, 1024], F32, tag="yT")
                for item in es_list:
                    _av_one(yT, h, item, LIVE_HALF[qs], qs)
                    yield
                _finish_section(yT, qs, h)

            def rr(*gens):
                """Round-robin drive generators to exhaustion."""
                live = list(gens)
                while live:
                    nxt = []
                    for g in live:
                        try:
                            next(g)
                            nxt.append(g)
                        except StopIteration:
                            pass
                    live = nxt

            def _finish_section(yT, qs, h):
                # evacuate yT (unnormalized) into its y_norm slot; the
                # denominator row goes out via ACT so DVE+ACT overlap and
                # the psy buffer frees fast.  The reciprocal chain's
                # replication+multiply for the PREVIOUS section is emitted
                # now (its rrow is long ready), so the PE never stalls on
                # the recip chain.
                mt = h // 2
                pb = 64 * (h % 2)
                dcp = pmisc.tile([1, 1024], F32, tag="dcp")
                nc.vector.tensor_copy(dcp[:], yT[HD : HD + 1, :])
                nc.vector.tensor_copy(
                    y_norm_qs[qs][pb : pb + 64, mt, :], yT[0:HD, :]
                )
                dT = pmisc.tile([128, 8], F32, tag="dT")
                nc.sync.dma_start(dT[:], dcp[:])
                rT = pmisc.tile([128, 8], BF, tag="rT")
                with nc.allow_low_precision(reason="softmax recip bf16"):
                    nc.vector.reciprocal(rT[:], dT[:])
                rrow = pmisc.tile([1, 1024], BF, tag="rrow")
                nc.sync.dma_start(rrow[:], rT[:])
                if pending_norm[0] is not None:
                    _emit_norm(pending_norm[0])
                pending_norm[0] = (rrow, pb, mt, qs)

            def outproj(qs):
                # stage the whole 512-query slab in SBUF, write it with ONE
                # DMA: avoids the ~2us per-DMA completion cost serializing
                # the PSUM ring at the tail.
                for sl in range(2):
                    qi = 2 * qs + sl
                    ost = pmisc.tile([128, 8, 512], BF, tag="ost")
                    for n in range(8):
                        po = pso_pool[0].tile([128, 512], F32, tag="out")
                        for c in range(2):
                            nc.tensor.matmul(
                                po[:],
                                wo[:, c, 128 * n : 128 * n + 128],
                                y_norm_qs[qs][:, c, 512 * sl : 512 * sl + 512],
                                start=(c == 0),
                                stop=(c == 1),
                            )
                        nc.vector.tensor_copy(ost[:, n, :], po[:])
                    nc.sync.dma_start(
                        outT[:, :, 512 * qi : 512 * qi + 512], ost[:]
                    )

            # ---------------- q/k projections (m-tile major) ----------------
            psy_pool = [None]
            pso_pool = [None]

            def proj_phase(w_sb, stg_list, b_sb, is_q, pool):
                # kc-outer over BOTH m-tiles: each staged chunk is fully
                # consumed (8 MMs, ~1.7us) as it lands, so the single
                # full-bandwidth input stream (~1.4us/chunk) keeps the PE
                # fed on the first pass.
                psums = [
                    pool.tile([128, 512], F32, tag="qk", name=f"p{mn}")
                    for mn in range(8)
                ]
                for kc in range(8):
                    for m in range(2):
                        for n in range(4):
                            nc.tensor.matmul(
                                psums[4 * m + n][:],
                                w_sb[:, kc, 128 * m : 128 * m + 128],
                                stg_list[kc][:, 512 * n : 512 * n + 512],
                                start=(kc == 0),
                                stop=(kc == 7),
                            )
                for m in range(2):
                    for n in range(4):
                        if is_q:
                            for sub in range(2):
                                pb = 64 * sub
                                nc.vector.tensor_scalar_add(
                                    qTz[2 * m + sub][
                                        pb : pb + 64, 512 * n : 512 * n + 512
                                    ],
                                    psums[4 * m + n][pb : pb + 64, :],
                                    b_sb[pb : pb + 64, m : m + 1],
                                )
                        else:
                            nc.vector.tensor_scalar_add(
                                kTt[m][:, 512 * n : 512 * n + 512],
                                psums[4 * m + n][:],
                                b_sb[:, m : m + 1],
                            )

            with tc.tile_pool(name="ppsum", bufs=8, space="PSUM") as pps:
                proj_phase(wq, stq, bq, True, pps)
                proj_phase(wk, stk, bk, False, pps)

            def v_gen(psv):
                for li in range(16):
                    pv = psv.tile([128, MPC], F32, tag="v")
                    for kc in range(8):
                        nc.tensor.matmul(
                            pv[:],
                            stv[kc][:, 128 * li : 128 * li + 128],
                            wv[:, kc, :],
                            start=(kc == 0),
                            stop=(kc == 7),
                        )
                    nc.vector.tensor_copy(
                        vxg[li // 4][:, li % 4, :, 0:HD],
                        pv[:].rearrange("p (h d) -> p h d", h=HPC),
                    )
                    yield

            es_store = {}

            def sc(qs, h, ki_lo=0, ki_hi=None):
                if ki_hi is None:
                    ki_hi = 8 * (qs + 1)
                lst = es_store.setdefault((qs, h), [])
                return scores_gen(qs, h, lst, ki_lo, ki_hi)

            def av(qs, h):
                return av_gen(qs, h, es_store[(qs, h)])

            # software pipeline: AV of section i-1 runs under the scores of
            # section i, so the scalar engine always has exp work queued
            # while the PE chews AV batches (and vice versa).
            with (
                tc.tile_pool(name="spsum", bufs=2, space="PSUM") as psc_,
                tc.tile_pool(name="ypsum", bufs=1, space="PSUM") as psy,
            ):
                psc = psc_
                psy_pool[0] = psy
                with tc.tile_pool(name="vpsum", bufs=2, space="PSUM") as psv:
                    rr(sc(0, 0), sc(0, 1), v_gen(psv))
                with tc.tile_pool(name="opsum", bufs=2, space="PSUM") as pso:
                    pso_pool[0] = pso
                    rr(av(0, 0), sc(0, 2))
                    rr(av(0, 1), sc(0, 3))
                    rr(av(0, 2), sc(1, 0, 0, 8))
                    rr(av(0, 3), sc(1, 0, 8, 16))
                    rr(av(1, 0), sc(1, 1))
                    outproj(0)
                    rr(av(1, 1), sc(1, 2))
                    rr(av(1, 2), sc(1, 3))
                    rr(av(1, 3))
                    _emit_norm(pending_norm[0])
                    pending_norm[0] = None
                    outproj(1)

    nc.finalize()
    return nc


def _host_tables(rel_emb: np.ndarray):
    """Per-core-group band tables; rel_emb is [NUM_BUCKETS, H]."""
    d = np.arange(4095)
    rp = d - 2047  # key - query
    buckets = _bucket(rp)
    bands = []
    c31s = []
    for h in range(H):
        vals = rel_emb[buckets, h].astype(np.float32)
        vals = np.where(rp > 0, np.float32(NEG), vals)  # causal mask
        band_pad = np.full(4223, NEG, np.float32)
        band_pad[:4095] = vals
        # BS[r, x] = band_pad[4095 + r - (x + 1152)]  (cols < 1152 unused)
        idx = 4095 + np.arange(128)[:, None] - np.arange(1152, 4096)[None, :]
        bands.append(band_pad[idx].astype(BF_NP))
        c31s.append(np.float32(rel_emb[31, h]))
    return bands, c31s


def _numpy_ref(query, key, value, attn_mask, key_padding_mask,
               Wq, bq, Wk, bk, Wv, bv, Wo, bo, rel_emb):
    """Exact numpy fallback for unexpected mask patterns."""
    q = (query @ Wq.T + bq).reshape(B, L, H, HD).transpose(0, 2, 1, 3)
    k = (key @ Wk.T + bk).reshape(B, L, H, HD).transpose(0, 2, 1, 3)
    v = (value @ Wv.T + bv).reshape(B, L, H, HD).transpose(0, 2, 1, 3)
    scores = np.einsum("bhqd,bhkd->bhqk", q, k) / math.sqrt(HD)
    rp = np.arange(L, dtype=np.int64)[None, :] - np.arange(L, dtype=np.int64)[:, None]
    rel = rel_emb[_bucket(rp)].transpose(2, 0, 1)
    scores = scores + rel[None]
    scores = np.where(attn_mask[None, None], scores, -np.inf)
    scores = np.where(key_padding_mask[:, None, None, :], scores, -np.inf)
    scores = scores - scores.max(-1, keepdims=True)
    e = np.exp(scores)
    attn = e / e.sum(-1, keepdims=True)
    out = np.einsum("bhqk,bhkd->bhqd", attn, v)
    out = out.transpose(0, 2, 1, 3).reshape(B, L, D)
    return (out @ Wo.T + bo).astype(np.float32)


def kernel(**inputs) -> np.ndarray:
    global _cached, last_results
    inp = {k: np.asarray(v) for k, v in inputs.items()}
    query, key, value = inp["query"], inp["key"], inp["value"]
    attn_mask, kpm = inp["attn_mask"], inp["key_padding_mask"]
    Wq, bq, Wk, bk = inp["Wq"], inp["bq"], inp["Wk"], inp["bk"]
    Wv, bv, Wo, bo = inp["Wv"], inp["bv"], inp["Wo"], inp["bo"]
    rel_emb = inp["rel_emb"]

    causal = np.array_equal(attn_mask, np.tril(np.ones((L, L), bool)))
    if not (causal and kpm.all()):
        return _numpy_ref(**inp)

    if _cached is None:
        _cached = _build()
    nc = _cached

    bands, c31s = _host_tables(rel_emb)
    ident = np.eye(128, dtype=BF_NP)

    def _rearr_w(w_slice):  # [MPC, D] row-major weights -> [128, 8, MPC]
        arr = np.ascontiguousarray(w_slice.T)  # [D, MPC]
        return arr.reshape(8, 128, MPC).transpose(1, 0, 2).astype(BF_NP)

    in_maps = []
    for c in range(N_CORES):
        b, hg = c // HPC, c % HPC
        rows = slice(MPC * hg, MPC * hg + MPC)
        heads = range(HPC * hg, HPC * hg + HPC)
        wo_c = np.ascontiguousarray(Wo[:, rows].T)  # [MPC, D]
        in_maps.append({
            "qT_in": np.ascontiguousarray(query[b].T).astype(BF_NP),
            "kT_in": np.ascontiguousarray(key[b].T).astype(BF_NP),
            "vT_in": np.ascontiguousarray(value[b].T).astype(BF_NP),
            "wq_in": _rearr_w(Wq[rows] / math.sqrt(HD)),
            "wk_in": _rearr_w(Wk[rows]),
            "wv_in": _rearr_w(Wv[rows]),
            "wo_in": wo_c.reshape(2, 128, D).transpose(1, 0, 2).astype(BF_NP),
            "bq_in": np.ascontiguousarray(
                (bq[rows] / math.sqrt(HD)).reshape(2, 128).T.astype(np.float32)
            ),
            "bk_in": np.ascontiguousarray(
                bk[rows].reshape(2, 128).T.astype(np.float32)
            ),
            "band_in": np.stack([bands[h] for h in heads]),
            "c31_in": np.tile(
                np.array([c31s[h] for h in heads], np.float32), (128, 1)
            ),
            "id_in": ident,
        })

    res = run_bass_kernel_spmd(nc, in_maps, list(range(N_CORES)))
    last_results = res

    bo_eff = (
        bo.astype(np.float64) + bv.astype(np.float64) @ Wo.T.astype(np.float64)
    )
    out = np.empty((B, L, D), np.float32)
    for b in range(B):
        acc = np.zeros((D, L), np.float64)
        for hg in range(HPC):
            part = res.results[b * HPC + hg]["outT"].astype(np.float64)
            acc += part.transpose(1, 0, 2).reshape(D, L)
        out[b] = (acc.T + bo_eff[None, :]).astype(np.float32)
    return out


# revision 43
# speedup vs baseline: 1.6078x; 1.0486x over previous
"""Bass/Trainium2 kernel for nn_MultiHeadAttention (T5-style rel-bias causal MHA).

Sharding: 8 cores = 2 batches x 4 head-groups (4 heads of 64 dims each).
Each core: projects q/k/v for its 256 proj rows (bf16 operands, fp32 PSUM),
runs causal attention with the T5 relative bias folded in as either a PSUM
band preload (near-diagonal blocks) or a constant exp-bias
(bucket-31-saturated blocks), and computes a partial out-projection.
Host sums the 4 bf16 partials per batch.

v2 layout: bf16 operands everywhere (halves HBM traffic vs f32r), m-tile
ordered q/k projection so attention sections for heads 0/1 start early
(their AV deferred until v-proj lands), qs-outer section order with the
qs0 out-projection overlapped under qs1 attention, and the scalar engine
kept free of DMA issue so it does nothing but exp.
"""
import math
import sys

sys.path.insert(0, "/opt/trn_rl_repo")

import numpy as np
import ml_dtypes

from concourse import bacc
import concourse.mybir as mybir
import concourse.tile as tile
from concourse.bass_utils import run_bass_kernel_spmd

F32 = mybir.dt.float32
F32R = mybir.dt.float32r
BF = mybir.dt.bfloat16
Exp = mybir.ActivationFunctionType.Exp
Copy = mybir.ActivationFunctionType.Copy
MUL = mybir.AluOpType.mult

B, L, D = 2, 2048, 1024
H, HD = 16, 64
NUM_BUCKETS, MAX_DISTANCE = 32, 128
HPC = 4  # heads per core
MPC = HPC * HD  # 256 proj rows per core
N_CORES = 8
NEG = -60.0  # additive mask value (exp(-60+s) == 0 in practice)
BF_NP = ml_dtypes.bfloat16
# per-qs: number of live 128-key blocks for each 512-query half
LIVE_HALF = {
    qs: [min(4 * (2 * qs + j + 1), 16) for j in (0, 1)] for qs in (0, 1)
}

last_results = None  # BassKernelResults of the most recent run (for profiling)
_cached = None


def _bucket(rp: np.ndarray) -> np.ndarray:
    """T5 relative position bucket, mirrors the reference exactly."""
    sign = (rp > 0).astype(np.int32)
    n = np.abs(rp)
    max_exact = NUM_BUCKETS // 2
    n_safe = np.maximum(n, 1).astype(np.float32)
    vil = max_exact + (
        np.log(n_safe / max_exact)
        / math.log(MAX_DISTANCE / max_exact)
        * (NUM_BUCKETS - max_exact)
    ).astype(np.int32)
    vil = np.minimum(vil, NUM_BUCKETS - 1)
    buckets = np.where(n < max_exact, n, vil) + sign * max_exact
    return np.clip(buckets, 0, NUM_BUCKETS - 1)


def _build():
    nc = bacc.Bacc(trn_type="TRN2")

    qT_in = nc.dram_tensor("qT_in", [D, L], BF, kind="ExternalInput")
    kT_in = nc.dram_tensor("kT_in", [D, L], BF, kind="ExternalInput")
    vT_in = nc.dram_tensor("vT_in", [D, L], BF, kind="ExternalInput")
    wq_in = nc.dram_tensor("wq_in", [128, 8, MPC], BF, kind="ExternalInput")
    wk_in = nc.dram_tensor("wk_in", [128, 8, MPC], BF, kind="ExternalInput")
    wv_in = nc.dram_tensor("wv_in", [128, 8, MPC], BF, kind="ExternalInput")
    wo_in = nc.dram_tensor("wo_in", [128, 2, D], BF, kind="ExternalInput")
    bq_in = nc.dram_tensor("bq_in", [128, 2], F32, kind="ExternalInput")
    bk_in = nc.dram_tensor("bk_in", [128, 2], F32, kind="ExternalInput")
    # band columns 0..1151 are never addressed (x0 >= 1152): trimmed
    band_in = nc.dram_tensor("band_in", [HPC, 128, 2944], BF, kind="ExternalInput")
    c31_in = nc.dram_tensor("c31_in", [128, HPC], F32, kind="ExternalInput")
    id_in = nc.dram_tensor("id_in", [128, 128], BF, kind="ExternalInput")
    # [p, n, l] layout: row (128n + p) of the [D, L] partial lives at [p, n, l]
    outT = nc.dram_tensor("outT", [128, 8, L], BF, kind="ExternalOutput")

    with tile.TileContext(nc) as tc:
        with (
            tc.tile_pool(name="res", bufs=1) as pr,
            tc.tile_pool(name="qkv", bufs=1) as pqkv,
            tc.tile_pool(name="stg", bufs=16) as pstg,
            tc.tile_pool(name="es", bufs=16) as pes,
            tc.tile_pool(name="misc", bufs=2) as pmisc,
        ):
            # ---- input DMAs ----
            # One serialized full-bandwidth stream on sync in consumption
            # order (qT, kT, vT) so the PE is never starved by fair-shared
            # HBM; weights/consts/bands ride gpsimd (SWDGE); scalar issues
            # nothing (kept pure-exp).
            # wq/bq lead the sync stream: HWDGE starts fast (~0.6us) while
            # the gpsimd SWDGE path takes ~3us to issue its first DMA.
            wq = pr.tile([128, 8, MPC], BF)
            nc.sync.dma_start(wq[:], wq_in[:])
            bq = pr.tile([128, 2], F32)
            nc.sync.dma_start(bq[:], bq_in[:])
            stq = []
            for kc in range(8):
                t = pstg.tile([128, L], BF, tag="stage", name=f"sq{kc}")
                nc.sync.dma_start(t[:], qT_in[128 * kc : 128 * kc + 128, :])
                stq.append(t)
            stk = []
            for kc in range(8):
                t = pstg.tile([128, L], BF, tag="stage", name=f"sk{kc}")
                nc.sync.dma_start(t[:], kT_in[128 * kc : 128 * kc + 128, :])
                stk.append(t)
            stv = []
            for kc in range(8):
                t = pstg.tile([128, L], BF, tag="stage", name=f"sv{kc}")
                nc.sync.dma_start(t[:], vT_in[128 * kc : 128 * kc + 128, :])
                stv.append(t)
            wk = pr.tile([128, 8, MPC], BF)
            nc.gpsimd.dma_start(wk[:], wk_in[:])
            bk = pr.tile([128, 2], F32)
            nc.gpsimd.dma_start(bk[:], bk_in[:])
            wv = pr.tile([128, 8, MPC], BF)
            nc.gpsimd.dma_start(wv[:], wv_in[:])
            c31 = pr.tile([128, HPC], F32)
            nc.gpsimd.dma_start(c31[:], c31_in[:])
            ident = pr.tile([128, 128], BF)
            nc.gpsimd.dma_start(ident[:], id_in[:])
            wo = pr.tile([128, 2, D], BF)
            nc.gpsimd.dma_start(wo[:], wo_in[:])
            bands = []
            for h in range(HPC):
                t = pr.tile([128, 2944], BF, name=f"band{h}")
                nc.gpsimd.dma_start(t[:], band_in[h])
                bands.append(t)

            # warm the ACT exp table early, off the critical path
            warm = pr.tile([1, 2], F32)
            nc.vector.memset(warm[:], 0.0)
            nc.scalar.activation(warm[:], warm[:], Exp)
            ones_v = pr.tile([1, HD], BF)
            nc.vector.memset(ones_v[:], 1.0)

            qTz = []
            for hh in range(HPC):
                t = pqkv.tile([128, L], BF, name=f"qtz{hh}")
                nc.vector.memset(t[:].bitcast(F32), 0)
                qTz.append(t)
            kTt = [pqkv.tile([128, L], BF, name=f"kt{mm}") for mm in range(2)]
            vxg = []
            for g in range(4):
                t = pqkv.tile([128, 4, HPC, HD + 1], BF, name=f"vx{g}")
                nc.vector.memset(t[:, :, :, HD], 1.0)
                vxg.append(t)
            y_norm_qs = [
                pqkv.tile([128, 2, 1024], BF, name=f"yn{qq}") for qq in range(2)
            ]

            # ---------------- attention section machinery ----------------
            pending_norm = [None]

            def _emit_norm(item):
                # PE-side replication of the reciprocal row + in-place
                # multiply; prep rides the score-psum ring.
                rrow, pb, mt, qsi = item
                prep = psc.tile([128, 1024], F32, tag="score", name="prep")
                nc.tensor.matmul(
                    prep[0:HD, :512], ones_v[:], rrow[:, :512],
                    start=True, stop=True,
                )
                nc.tensor.matmul(
                    prep[0:HD, 512:], ones_v[:], rrow[:, 512:],
                    start=True, stop=True,
                )
                nc.vector.tensor_tensor(
                    y_norm_qs[qsi][pb : pb + 64, mt, :],
                    y_norm_qs[qsi][pb : pb + 64, mt, :],
                    prep[0:HD, :],
                    MUL,
                )

            def scores_gen(qs, h, es_list, ki_lo, ki_hi):
                """Generator: emit preload+score MMs and exp for section
                (qs, h), one ki per yield; AV is emitted separately."""
                mt = h // 2
                q0 = 1024 * qs
                for ki in range(ki_lo, ki_hi):
                    const_blk = 128 * ki <= q0 - 240
                    live_half = LIVE_HALF[qs]
                    halves = [j for j in (0, 1) if ki < live_half[j]]
                    # causal staircase: query-columns below the diagonal are
                    # dead; narrow the moving operand to the live range.
                    c0s = {
                        j: max(0, 128 * ki - (q0 + 512 * j)) for j in halves
                    }
                    sp = psc.tile([128, 1024], F32, tag="score")
                    for j in halves:
                        a = 512 * j + c0s[j]
                        b = 512 * j + 512
                        if not const_blk:
                            x0 = 2048 - 128 * ki + q0 - 1152 + a
                            nc.tensor.matmul(
                                sp[:, a:b],
                                ident[:],
                                bands[h][:, x0 : x0 + (b - a)],
                                start=True,
                                stop=False,
                            )
                        nc.tensor.matmul(
                            sp[:, a:b],
                            kTt[mt][:, 128 * ki : 128 * ki + 128],
                            qTz[h][:, q0 + a : q0 + b],
                            start=const_blk,
                            stop=True,
                        )
                    es = pes.tile([128, 1024], BF, tag="es")
                    bias = c31[:, h : h + 1] if const_blk else 0.0
                    # live ranges of the two halves are always contiguous
                    lo = min(512 * j + c0s[j] for j in halves)
                    hi = 512 * halves[-1] + 512
                    nc.scalar.activation(
                        es[:, lo:hi], sp[:, lo:hi], Exp, bias=bias
                    )
                    es_list.append((es, halves, ki))
                    yield

            def _av_one(yT, h, item, live_half, qs):
                es, halves, ki = item
                q0 = 1024 * qs
                for j in halves:
                    a = 512 * j + max(0, 128 * ki - (q0 + 512 * j))
                    b = 512 * j + 512
                    nc.tensor.matmul(
                        yT[:, a:b],
                        vxg[ki // 4][:, ki % 4, h, :],
                        es[:, a:b],
                        start=(ki == 0),
                        stop=(ki == live_half[j] - 1),
                    )

            def av_gen(qs, h, es_list, last=False):
                yT = psy_pool[0].tile([HD + 1, 1024], F32, tag="yT")
                for item in es_list:
                    _av_one(yT, h, item, LIVE_HALF[qs], qs)
                    yield
                _finish_section(yT, qs, h, last)

            def rr(*gens):
                """Round-robin drive generators to exhaustion."""
                live = list(gens)
                while live:
                    nxt = []
                    for g in live:
                        try:
                            next(g)
                            nxt.append(g)
                        except StopIteration:
                            pass
                    live = nxt

            def _finish_section(yT, qs, h, last=False):
                # evacuate yT (unnormalized) into its y_norm slot; the
                # denominator row goes out via ACT so DVE+ACT overlap and
                # the psy buffer frees fast.  The reciprocal chain's
                # replication+multiply for the PREVIOUS section is emitted
                # now (its rrow is long ready), so the PE never stalls on
                # the recip chain.
                mt = h // 2
                pb = 64 * (h % 2)
                dcp = pmisc.tile([1, 1024], F32, tag="dcp")
                if last:
                    # final section: denominator row via ACT so its recip
                    # chain starts in parallel with the DVE y-copy
                    nc.scalar.activation(dcp[:], yT[HD : HD + 1, :], Copy)
                else:
                    nc.vector.tensor_copy(dcp[:], yT[HD : HD + 1, :])
                nc.vector.tensor_copy(
                    y_norm_qs[qs][pb : pb + 64, mt, :], yT[0:HD, :]
                )
                dT = pmisc.tile([128, 8], F32, tag="dT")
                nc.sync.dma_start(dT[:], dcp[:])
                rT = pmisc.tile([128, 8], BF, tag="rT")
                with nc.allow_low_precision(reason="softmax recip bf16"):
                    nc.vector.reciprocal(rT[:], dT[:])
                rrow = pmisc.tile([1, 1024], BF, tag="rrow")
                nc.sync.dma_start(rrow[:], rT[:])
                if pending_norm[0] is not None:
                    _emit_norm(pending_norm[0])
                pending_norm[0] = (rrow, pb, mt, qs)

            def outproj(qs):
                # stage the whole 512-query slab in SBUF, write it with ONE
                # DMA: avoids the ~2us per-DMA completion cost serializing
                # the PSUM ring at the tail.
                for sl in range(2):
                    qi = 2 * qs + sl
                    ost = pmisc.tile([128, 8, 512], BF, tag="ost")
                    for n in range(8):
                        po = pso_pool[0].tile([128, 512], F32, tag="out")
                        for c in range(2):
                            nc.tensor.matmul(
                                po[:],
                                wo[:, c, 128 * n : 128 * n + 128],
                                y_norm_qs[qs][:, c, 512 * sl : 512 * sl + 512],
                                start=(c == 0),
                                stop=(c == 1),
                            )
                        # alternate DVE/ACT for the PSUM evacuation so the
                        # pso ring is never evac-paced
                        if n % 2 == 0:
                            nc.vector.tensor_copy(ost[:, n, :], po[:])
                        else:
                            nc.scalar.activation(ost[:, n, :], po[:], Copy)
                        if n == 3:
                            nc.sync.dma_start(
                                outT[:, 0:4, 512 * qi : 512 * qi + 512],
                                ost[:, 0:4, :],
                            )
                    nc.sync.dma_start(
                        outT[:, 4:8, 512 * qi : 512 * qi + 512],
                        ost[:, 4:8, :],
                    )

            # ---------------- q/k projections (m-tile major) ----------------
            psy_pool = [None]
            pso_pool = [None]

            def proj_phase(w_sb, stg_list, b_sb, is_q, pool):
                # kc-outer over BOTH m-tiles: each staged chunk is fully
                # consumed (8 MMs, ~1.7us) as it lands, so the single
                # full-bandwidth input stream (~1.4us/chunk) keeps the PE
                # fed on the first pass.
                psums = [
                    pool.tile([128, 512], F32, tag="qk", name=f"p{mn}")
                    for mn in range(8)
                ]
                for kc in range(8):
                    for m in range(2):
                        for n in range(4):
                            nc.tensor.matmul(
                                psums[4 * m + n][:],
                                w_sb[:, kc, 128 * m : 128 * m + 128],
                                stg_list[kc][:, 512 * n : 512 * n + 512],
                                start=(kc == 0),
                                stop=(kc == 7),
                            )
                # alternate DVE/ACT so the evacuation burst after kc==7
                # doesn't serialize on one engine (ACT is idle pre-attention)
                for m in range(2):
                    for n in range(4):
                        if is_q:
                            for sub in range(2):
                                pb = 64 * sub
                                dst = qTz[2 * m + sub][
                                    pb : pb + 64, 512 * n : 512 * n + 512
                                ]
                                src = psums[4 * m + n][pb : pb + 64, :]
                                bias = b_sb[pb : pb + 64, m : m + 1]
                                if (n + sub) % 2 == 0:
                                    nc.vector.tensor_scalar_add(dst, src, bias)
                                else:
                                    nc.scalar.add(dst, src, bias)
                        else:
                            dst = kTt[m][:, 512 * n : 512 * n + 512]
                            src = psums[4 * m + n][:]
                            bias = b_sb[:, m : m + 1]
                            if n % 2 == 0:
                                nc.vector.tensor_scalar_add(dst, src, bias)
                            else:
                                nc.scalar.add(dst, src, bias)

            with tc.tile_pool(name="ppsum", bufs=8, space="PSUM") as pps:
                proj_phase(wq, stq, bq, True, pps)
                proj_phase(wk, stk, bk, False, pps)

            def v_gen(psv):
                for li in range(16):
                    pv = psv.tile([128, MPC], F32, tag="v")
                    for kc in range(8):
                        nc.tensor.matmul(
                            pv[:],
                            stv[kc][:, 128 * li : 128 * li + 128],
                            wv[:, kc, :],
                            start=(kc == 0),
                            stop=(kc == 7),
                        )
                    nc.vector.tensor_copy(
                        vxg[li // 4][:, li % 4, :, 0:HD],
                        pv[:].rearrange("p (h d) -> p h d", h=HPC),
                    )
                    yield

            es_store = {}

            def sc(qs, h, ki_lo=0, ki_hi=None):
                if ki_hi is None:
                    ki_hi = 8 * (qs + 1)
                lst = es_store.setdefault((qs, h), [])
                return scores_gen(qs, h, lst, ki_lo, ki_hi)

            def av(qs, h):
                return av_gen(qs, h, es_store[(qs, h)])

            # software pipeline: AV of section i-1 runs under the scores of
            # section i, so the scalar engine always has exp work queued
            # while the PE chews AV batches (and vice versa).
            with (
                tc.tile_pool(name="spsum", bufs=2, space="PSUM") as psc_,
                tc.tile_pool(name="ypsum", bufs=1, space="PSUM") as psy,
            ):
                psc = psc_
                psy_pool[0] = psy
                with tc.tile_pool(name="vpsum", bufs=2, space="PSUM") as psv:
                    rr(sc(0, 0), sc(0, 1), v_gen(psv))
                with tc.tile_pool(name="opsum", bufs=2, space="PSUM") as pso:
                    pso_pool[0] = pso
                    rr(av(0, 0), sc(0, 2))
                    rr(av(0, 1), sc(0, 3))
                    rr(av(0, 2), sc(1, 0, 0, 8))
                    rr(av(0, 3), sc(1, 0, 8, 16))
                    rr(av(1, 0), sc(1, 1))
                    outproj(0)
                    rr(av(1, 1), sc(1, 2))
                    rr(av(1, 2), sc(1, 3))
                    rr(av_gen(1, 3, es_store[(1, 3)], last=True))
                    _emit_norm(pending_norm[0])
                    pending_norm[0] = None
                    outproj(1)

    nc.finalize()
    return nc


def _host_tables(rel_emb: np.ndarray):
    """Per-core-group band tables; rel_emb is [NUM_BUCKETS, H]."""
    d = np.arange(4095)
    rp = d - 2047  # key - query
    buckets = _bucket(rp)
    bands = []
    c31s = []
    for h in range(H):
        vals = rel_emb[buckets, h].astype(np.float32)
        vals = np.where(rp > 0, np.float32(NEG), vals)  # causal mask
        band_pad = np.full(4223, NEG, np.float32)
        band_pad[:4095] = vals
        # BS[r, x] = band_pad[4095 + r - (x + 1152)]  (cols < 1152 unused)
        idx = 4095 + np.arange(128)[:, None] - np.arange(1152, 4096)[None, :]
        bands.append(band_pad[idx].astype(BF_NP))
        c31s.append(np.float32(rel_emb[31, h]))
    return bands, c31s


def _numpy_ref(query, key, value, attn_mask, key_padding_mask,
               Wq, bq, Wk, bk, Wv, bv, Wo, bo, rel_emb):
    """Exact numpy fallback for unexpected mask patterns."""
    q = (query @ Wq.T + bq).reshape(B, L, H, HD).transpose(0, 2, 1, 3)
    k = (key @ Wk.T + bk).reshape(B, L, H, HD).transpose(0, 2, 1, 3)
    v = (value @ Wv.T + bv).reshape(B, L, H, HD).transpose(0, 2, 1, 3)
    scores = np.einsum("bhqd,bhkd->bhqk", q, k) / math.sqrt(HD)
    rp = np.arange(L, dtype=np.int64)[None, :] - np.arange(L, dtype=np.int64)[:, None]
    rel = rel_emb[_bucket(rp)].transpose(2, 0, 1)
    scores = scores + rel[None]
    scores = np.where(attn_mask[None, None], scores, -np.inf)
    scores = np.where(key_padding_mask[:, None, None, :], scores, -np.inf)
    scores = scores - scores.max(-1, keepdims=True)
    e = np.exp(scores)
    attn = e / e.sum(-1, keepdims=True)
    out = np.einsum("bhqk,bhkd->bhqd", attn, v)
    out = out.transpose(0, 2, 1, 3).reshape(B, L, D)
    return (out @ Wo.T + bo).astype(np.float32)


def kernel(**inputs) -> np.ndarray:
    global _cached, last_results
    inp = {k: np.asarray(v) for k, v in inputs.items()}
    query, key, value = inp["query"], inp["key"], inp["value"]
    attn_mask, kpm = inp["attn_mask"], inp["key_padding_mask"]
    Wq, bq, Wk, bk = inp["Wq"], inp["bq"], inp["Wk"], inp["bk"]
    Wv, bv, Wo, bo = inp["Wv"], inp["bv"], inp["Wo"], inp["bo"]
    rel_emb = inp["rel_emb"]

    causal = np.array_equal(attn_mask, np.tril(np.ones((L, L), bool)))
    if not (causal and kpm.all()):
        return _numpy_ref(**inp)

    if _cached is None:
        _cached = _build()
    nc = _cached

    bands, c31s = _host_tables(rel_emb)
    ident = np.eye(128, dtype=BF_NP)

    def _rearr_w(w_slice):  # [MPC, D] row-major weights -> [128, 8, MPC]
        arr = np.ascontiguousarray(w_slice.T)  # [D, MPC]
        return arr.reshape(8, 128, MPC).transpose(1, 0, 2).astype(BF_NP)

    in_maps = []
    for c in range(N_CORES):
        b, hg = c // HPC, c % HPC
        rows = slice(MPC * hg, MPC * hg + MPC)
        heads = range(HPC * hg, HPC * hg + HPC)
        wo_c = np.ascontiguousarray(Wo[:, rows].T)  # [MPC, D]
        in_maps.append({
            "qT_in": np.ascontiguousarray(query[b].T).astype(BF_NP),
            "kT_in": np.ascontiguousarray(key[b].T).astype(BF_NP),
            "vT_in": np.ascontiguousarray(value[b].T).astype(BF_NP),
            "wq_in": _rearr_w(Wq[rows] / math.sqrt(HD)),
            "wk_in": _rearr_w(Wk[rows]),
            "wv_in": _rearr_w(Wv[rows]),
            "wo_in": wo_c.reshape(2, 128, D).transpose(1, 0, 2).astype(BF_NP),
            "bq_in": np.ascontiguousarray(
                (bq[rows] / math.sqrt(HD)).reshape(2, 128).T.astype(np.float32)
            ),
            "bk_in": np.ascontiguousarray(
                bk[rows].reshape(2, 128).T.astype(np.float32)
            ),
            "band_in": np.stack([bands[h] for h in heads]),
            "c31_in": np.tile(
                np.array([c31s[h] for h in heads], np.float32), (128, 1)
            ),
            "id_in": ident,
        })

    res = run_bass_kernel_spmd(nc, in_maps, list(range(N_CORES)))
    last_results = res

    bo_eff = (
        bo.astype(np.float64) + bv.astype(np.float64) @ Wo.T.astype(np.float64)
    )
    out = np.empty((B, L, D), np.float32)
    for b in range(B):
        acc = np.zeros((D, L), np.float64)
        for hg in range(HPC):
            part = res.results[b * HPC + hg]["outT"].astype(np.float64)
            acc += part.transpose(1, 0, 2).reshape(D, L)
        out[b] = (acc.T + bo_eff[None, :]).astype(np.float32)
    return out
